# revision 1
# baseline (speedup 1.0000x reference)
"""Causal multi-head attention (B=4, T=2048, D=1024, H=16) on 8 NeuronCores.

Sharding:
  stage 1 (QKV proj + attention): core c -> batch c//2, head-group c%2
    (8 of 16 heads, 512 of 1024 channels). Data-parallel on B, tensor-
    parallel on heads.
  stage 2 (output projection): one 8-rank AllToAll re-shards attention
    output to (all 4 batches x 256-token t-slice) per core, then each core
    computes out = attn_out @ W_O.T for its 1024 rows. No reduction needed.

All heavy matmuls run in fp32r (full PE rate, ~19-bit mantissa). exp runs on
the scalar engine reading PSUM directly with the softmax scale fused; the
softmax denominator comes for free as a 65th output row of the PV matmul
(V augmented with a ones column). Causal masking multiplies diagonal-block
probabilities by precomputed 0/1 masks.

The t-chunk loop interleaves projections with attention: after projecting
chunk tc, all k-tiles needed by q-chunk tc exist, so attention for q-chunk tc
runs while the next chunk's projections stream — keeping PE busy during the
ACT-heavy attention phase.
"""
import numpy as np

import concourse.bass as bass
import concourse.mybir as mybir
import concourse.tile as tile
from concourse.bass_utils import run_bass_kernel_spmd

F32 = mybir.dt.float32
F32R = mybir.dt.float32r

P = 128
B, T, D = 4, 2048, 1024
H, HD = 16, 64
NCORES = 8
CH = D // 2          # channels per core (8 heads)
NHP = 4              # head pairs per core
NKT = T // P         # 16 k-tiles
NQC = T // 512       # 4 q-chunks
NIT = D // P         # 8 input-dim tiles
TS256 = 256          # t-slice per core per batch in stage 2


def _split_multiwaits(nc) -> int:
    """walrus here rejects >1 sem wait per instruction; split extras into
    wait-only NoOps on the same engine."""
    nsplit = 0
    for f in nc.m.functions:
        for bb in f.blocks:
            if not any(
                i.sync_info is not None and i.sync_info.on_wait is not None
                and len(i.sync_info.on_wait) > 1 for i in bb.instructions
            ):
                continue
            new_list = []
            for inst in bb.instructions:
                si = inst.sync_info
                if si is not None and si.on_wait is not None and len(si.on_wait) > 1:
                    waits = list(si.on_wait)
                    for k, w in enumerate(waits[:-1]):
                        n = mybir.InstNoOp(
                            name=f"{inst.name}-wsplit{k}", ins=[], outs=[])
                        n.engine = inst.engine
                        n.sync_info = mybir.SyncInfo(on_wait=[w], on_update=[])
                        new_list.append(n)
                        nsplit += 1
                    inst.sync_info = mybir.SyncInfo(
                        on_wait=[waits[-1]], on_update=list(si.on_update or []))
                new_list.append(inst)
            bb.instructions = new_list
    return nsplit


def _build_nc(sim: bool = False, mask_mode: str = "dve"):
    nc = bass.Bass("TRN2", target_bir_lowering=False, debug=False,
                   num_devices=NCORES)
    xt_d = nc.dram_tensor("xt", [D, T], F32R, kind="ExternalInput").ap()
    wq_d = nc.dram_tensor("wq", [D, CH], F32R, kind="ExternalInput").ap()
    wk_d = nc.dram_tensor("wk", [D, CH], F32R, kind="ExternalInput").ap()
    wv_d = nc.dram_tensor("wv", [D, CH], F32R, kind="ExternalInput").ap()
    wo_d = nc.dram_tensor("wo", [D, D], F32R, kind="ExternalInput").ap()
    ones_d = nc.dram_tensor("ones", [P, NKT * NHP * 2], F32R,
                            kind="ExternalInput").ap()
    out_d = nc.dram_tensor("out", [B, 2, P, D], F32, kind="ExternalOutput").ap()
    a2a_in0 = nc.dram_tensor("a2a_in0", [NCORES, CH, P], F32R).ap()
    a2a_out0 = nc.dram_tensor("a2a_out0", [NCORES, CH, P], F32R).ap()
    a2a_in1 = nc.dram_tensor("a2a_in1", [NCORES, CH, P], F32R).ap()
    a2a_out1 = nc.dram_tensor("a2a_out1", [NCORES, CH, P], F32R).ap()

    scale = float(1.0 / np.sqrt(HD))

    with tile.TileContext(nc) as tc:
        with (
            tc.tile_pool(name="persist", bufs=1) as persist,
        ):
            # ---- persistent SBUF tensors -------------------------------
            kt_s = persist.tile([P, NHP, T], F32R)    # K^T  (channels, k)
            va = persist.tile([P, NKT, NHP, 2, HD + 1], F32R)  # V | ones

            with (
                tc.tile_pool(name="wpool", bufs=1) as wpool,
                tc.tile_pool(name="xpool", bufs=1) as xpool,
                tc.tile_pool(name="ob_pool", bufs=2) as ob_pool,
                tc.tile_pool(name="qpool", bufs=2) as qpool,
                tc.tile_pool(name="ao_pool", bufs=2) as ao_pool,
                tc.tile_pool(name="mpool", bufs=1) as mpool,
                tc.tile_pool(name="pt_pool", bufs=4) as pt_pool,
                tc.tile_pool(name="nrm_pool", bufs=1) as nrm_pool,
                tc.tile_pool(name="ppool", bufs=2, space="PSUM") as ppool,
                tc.tile_pool(name="ps_s", bufs=2, space="PSUM") as ps_s,
                tc.tile_pool(name="ps_pv", bufs=1, space="PSUM") as ps_pv,
            ):
                wq = wpool.tile([P, NIT, CH], F32R)
                wk = wpool.tile([P, NIT, CH], F32R)
                wv = wpool.tile([P, NIT, CH], F32R)
                xt_r = xt_d.rearrange("(i p) t -> p i t", p=P)
                xtc0 = xpool.tile([P, NIT, 512], F32R, tag="xtc")
                for it in range(NIT):
                    nc.sync.dma_start(xtc0[:, it], xt_r[:, it, 0:512])
                    nc.sync.dma_start(wv[:, it], wv_d.rearrange(
                        "(i p) o -> p i o", p=P)[:, it])
                for it in range(NIT):
                    nc.sync.dma_start(wq[:, it], wq_d.rearrange(
                        "(i p) o -> p i o", p=P)[:, it])
                    nc.sync.dma_start(wk[:, it], wk_d.rearrange(
                        "(i p) o -> p i o", p=P)[:, it])

                ones64 = mpool.tile([P, 64], F32R)
                nc.sync.dma_start(ones64[:], ones_d[:, 0:64])
                # fill the V|ones denominator column via one broadcast copy
                # (a strided DMA here would be thousands of 4B descriptors)
                nc.scalar.copy(
                    va[:, :, :, :, HD],
                    ones64[:, 0:1].to_broadcast((P, NKT, NHP, 2)))
                masks = []
                if mask_mode == "dve":
                    for i in range(4):
                        m = mpool.tile([P, 512], mybir.dt.bfloat16,
                                       tag=f"mask{i}")
                        nc.gpsimd.memset(m[:], 1.0)
                        nc.gpsimd.affine_select(
                            out=m[:], in_=m[:],
                            compare_op=mybir.AluOpType.is_ge,
                            fill=0.0, base=-P * i, channel_multiplier=-1,
                            pattern=[[1, 512]])
                        masks.append(m)

                # pending projection psum-groups of the NEXT chunk, emitted
                # as PE filler work inside the attention kt loops
                pending = []
                normtail = []

                filler_acc = [0.0]

                def emit_fillers(remaining_units):
                    # proportional pacing: spread the queue across the whole
                    # remaining stage instead of draining it in the first
                    # len(pending) units (late ACT-bound units idle PE)
                    if not pending:
                        return
                    filler_acc[0] += len(pending) / max(1, remaining_units)
                    while filler_acc[0] >= 1.0 and pending:
                        filler_acc[0] -= 1.0
                        pending.pop(0)()

                def project(tc4, xtc=None):
                    """Queue QKV projection psum-groups for t-chunk tc4.
                    Returns the Q^T chunk tile; the groups themselves are
                    emitted later as PE filler inside attention."""
                    if xtc is None:
                        xtc = xpool.tile([P, NIT, 512], F32R, tag="xtc")
                        for it in range(NIT):
                            nc.sync.dma_start(
                                xtc[:, it],
                                xt_r[:, it, tc4 * 512:(tc4 + 1) * 512])
                    qtc = qpool.tile([P, NHP, 512], F32R, tag="qtc")

                    def qk_group(w, dst, dsl, ot):
                        def g():
                            ps = ppool.tile([P, 512], F32, tag="proj")
                            for it in range(NIT):
                                nc.tensor.matmul(
                                    ps[:], w[:, it, ot * P:(ot + 1) * P],
                                    xtc[:, it], start=(it == 0),
                                    stop=(it == NIT - 1))
                            nc.vector.tensor_copy(dst[:, ot, dsl], ps[:])
                        return g

                    def v_group(tt4):
                        def g():
                            ps = ppool.tile([P, 512], F32, tag="proj")
                            for it in range(NIT):
                                nc.tensor.matmul(
                                    ps[:], xtc[:, it, tt4 * P:(tt4 + 1) * P],
                                    wv[:, it], start=(it == 0),
                                    stop=(it == NIT - 1))
                            nc.vector.tensor_copy(
                                va[:, tc4 * 4 + tt4, :, :, 0:HD],
                                ps[:].rearrange("p (hp h d) -> p hp h d",
                                                hp=NHP, h=2))
                        return g

                    if tc4 == 0:
                        # V first: wv+x arrive first and the four V groups
                        # run it-major across four concurrent psums, so each
                        # arriving (x, wv) DMA chunk feeds 4 matmuls instead
                        # of 1 during the DMA-bound startup ramp
                        def v_block0():
                            pss = [
                                ppool.tile([P, 512], F32, tag="proj",
                                           name="v0ps0"),
                                ppool.tile([P, 512], F32, tag="proj",
                                           name="v0ps1"),
                                ps_s.tile([P, 512], F32, tag="s2",
                                          name="v0ps2"),
                                ps_s.tile([P, 512], F32, tag="s2",
                                          name="v0ps3"),
                            ]
                            for it in range(NIT):
                                for tt4 in range(4):
                                    nc.tensor.matmul(
                                        pss[tt4][:],
                                        xtc[:, it, tt4 * P:(tt4 + 1) * P],
                                        wv[:, it], start=(it == 0),
                                        stop=(it == NIT - 1))
                            for tt4 in range(4):
                                nc.vector.tensor_copy(
                                    va[:, tt4, :, :, 0:HD],
                                    pss[tt4][:].rearrange(
                                        "p (hp h d) -> p hp h d",
                                        hp=NHP, h=2))
                        pending.append(v_block0)
                        for ot in range(NHP):
                            pending.append(qk_group(wq, qtc, slice(0, 512), ot))
                            pending.append(qk_group(
                                wk, kt_s,
                                slice(tc4 * 512, (tc4 + 1) * 512), ot))
                    else:
                        for ot in range(NHP):
                            pending.append(qk_group(wq, qtc, slice(0, 512), ot))
                        for ot in range(NHP):
                            pending.append(qk_group(
                                wk, kt_s, slice(tc4 * 512, (tc4 + 1) * 512), ot))
                        for tt4 in range(4):
                            pending.append(v_group(tt4))
                    return qtc

                def attend(hp, qc, qtc, aoq):
                    """Attention for head-pair hp, q-chunk qc. kt loop is
                    software-pipelined: QK(kt+1) issues before PV(kt) so PE
                    isn't stalled behind the exp of the current tile."""
                    nkt = 4 * (qc + 1)
                    pva = ps_pv.tile([HD + 1, 512], F32, tag="pva")
                    pvb = ps_pv.tile([HD + 1, 512], F32, tag="pvb")
                    s2s = {}
                    pts = {}

                    def qk(kt):
                        ksl = slice(kt * P, (kt + 1) * P)
                        f0 = max(0, kt - 4 * qc) * P  # first visible q column
                        s2 = ps_s.tile([P, 1024], F32, tag="s2")
                        nc.tensor.matmul(s2[:, f0:512], kt_s[0:64, hp, ksl],
                                         qtc[0:64, hp, f0:],
                                         start=True, stop=True)
                        nc.tensor.matmul(s2[:, 512 + f0:1024],
                                         kt_s[64:128, hp, ksl],
                                         qtc[64:128, hp, f0:],
                                         start=True, stop=True)
                        s2s[kt] = s2

                    def softmax_pv(kt, remaining):
                        s2 = s2s.pop(kt)
                        pt = pt_pool.tile([P, 2, 512], F32R, tag="pt")
                        di = kt - 4 * qc
                        if mask_mode == "gp":
                            f0 = max(0, di) * P
                            s2v = s2[:].rearrange("p (a b) -> p a b", a=2)
                            nc.scalar.activation(
                                pt[:, :, f0:], s2v[:, :, f0:],
                                mybir.ActivationFunctionType.Exp, scale=scale)
                            if di >= 0:
                                # causal: keep q >= k, zero the rest (incl the
                                # [0:f0) region the restricted exp skipped)
                                nc.gpsimd.affine_select(
                                    out=pt[:], in_=pt[:],
                                    compare_op=mybir.AluOpType.is_ge,
                                    fill=0.0, base=-P * di,
                                    channel_multiplier=-1,
                                    pattern=[[0, 2], [1, 512]])
                        else:
                            # diagonal blocks: only columns >= f0 are causally
                            # visible; exp, mask, and PV all restrict to them
                            # (kt==0 is always full-width, initializing every
                            # PSUM column of the PV accumulators)
                            f0 = max(0, di) * P
                            if f0 > 0:
                                s2v = s2[:].rearrange("p (a b) -> p a b", a=2)
                                nc.scalar.activation(
                                    pt[:, :, f0:], s2v[:, :, f0:],
                                    mybir.ActivationFunctionType.Exp,
                                    scale=scale)
                            else:
                                nc.scalar.activation(
                                    pt[:].rearrange("p a b -> p (a b)"), s2[:],
                                    mybir.ActivationFunctionType.Exp,
                                    scale=scale)
                            if di >= 0:
                                nc.vector.tensor_mul(
                                    pt[:, :, f0:], pt[:, :, f0:],
                                    masks[di][:, None, f0:].to_broadcast(
                                        (P, 2, 512 - f0)))
                        f0 = max(0, di) * P
                        nc.tensor.matmul(pva[:, f0:], va[:, kt, hp, 0],
                                         pt[:, 0, f0:],
                                         start=(kt == 0), stop=(kt == nkt - 1))
                        nc.tensor.matmul(pvb[:, f0:], va[:, kt, hp, 1],
                                         pt[:, 1, f0:],
                                         start=(kt == 0), stop=(kt == nkt - 1))
                        if kt >= 2 and normtail:
                            normtail.pop(0)()
                        emit_fillers(remaining)

                    qk(0)
                    for kt in range(1, nkt):
                        qk(kt)
                        softmax_pv(kt - 1, (nkt - kt) + (NHP - 1 - hp) * nkt)
                    softmax_pv(nkt - 1, 1 + (NHP - 1 - hp) * nkt)

                    # copy PV accumulators out of PSUM fast (frees banks);
                    # defer the recip->broadcast->scale tail into the next
                    # head-pair's kt loop so PE never stalls behind it
                    pvs = nrm_pool.tile([P, 2, 512], F32, tag="pvs")
                    nc.vector.tensor_copy(pvs[0:65, 0], pva[:])
                    if hp == NHP - 1 and qc in (1, 3):
                        # stage-final tail gates a collective launch and runs
                        # with ACT idle: split the copies across engines to
                        # shorten the serial chain
                        nc.scalar.copy(pvs[0:65, 1], pvb[:])
                    else:
                        nc.vector.tensor_copy(pvs[0:65, 1], pvb[:])
                    rden = nrm_pool.tile([P, 2, 512], F32R, tag="rden")
                    with nc.allow_low_precision("f32r softmax denominators"):
                        nc.vector.reciprocal(rden[64:65, 0], pvs[64:65, 0])
                        nc.vector.reciprocal(rden[64:65, 1], pvs[64:65, 1])

                    def tail(hp=hp, pvs=pvs, rden=rden):
                        rba = ppool.tile([64, 512], F32, tag="proj")
                        rbb = ppool.tile([64, 512], F32, tag="proj")
                        nc.tensor.matmul(rba[:], ones64[64:65, :],
                                         rden[64:65, 0], start=True, stop=True)
                        nc.tensor.matmul(rbb[:], ones64[64:65, :],
                                         rden[64:65, 1], start=True, stop=True)
                        nc.vector.tensor_mul(aoq[0:64, hp], pvs[0:64, 0],
                                             rba[:])
                        nc.vector.tensor_mul(aoq[64:128, hp], pvs[0:64, 1],
                                             rbb[:])
                        # ship this head-pair's slice to the exchange buffer
                        # immediately so the collective's inputs aren't gated
                        # on one bulk DMA burst at stage end
                        nc.sync.dma_start(
                            a2a_r[qc // 2][:, hp, (qc % 2) * 4:(qc % 2) * 4 + 4],
                            aoq[:, hp].rearrange("p (j t) -> p j t", j=4))
                    normtail.append(tail)

                # interleaved: project chunk tc, then attention q-chunk tc,
                # streaming each finished chunk into the re-shard buffers.
                # stage-2 row owner of q = m*1024 + j*128 + p is core j, so
                # the first collective can fire once q < 1024 is done.
                a2a_r = [a.rearrange("j (hp p) t -> p hp j t", p=P)
                         for a in (a2a_in0, a2a_in1)]

                def emit_collective(m):
                    cin = (a2a_in0, a2a_in1)[m]
                    cout = (a2a_out0, a2a_out1)[m]
                    if sim:
                        nc.sync.dma_start(cout, cin)
                    else:
                        nc.gpsimd.collective_compute(
                            "AllToAll", mybir.AluOpType.bypass,
                            replica_groups=[list(range(NCORES))],
                            ins=[cin], outs=[cout])

                qtc = project(0, xtc=xtc0)
                while pending:
                    pending.pop(0)()
                for tc4 in range(NQC):
                    if tc4 + 1 < NQC:
                        next_qtc = project(tc4 + 1)  # queued as fillers
                    aoq = ao_pool.tile([P, NHP, 512], F32R, tag="aoq")
                    for hp in range(NHP):
                        attend(hp, tc4, qtc, aoq)
                    if tc4 in (1, 3):
                        # drains are only load-bearing before a collective
                        # launch; elsewhere tails/fillers spill into the next
                        # stage's kt loops for smoother boundaries
                        while normtail:
                            normtail.pop(0)()
                        while pending:
                            pending.pop(0)()
                    if tc4 == 1:
                        emit_collective(0)
                    if tc4 == 2:
                        # chunk-3 projections are queued, so the wq/wk pool
                        # slots retire after them; reuse them for W_O and
                        # queue the m=0 output projection as qc3 filler work
                        # (its AllToAll finished during qc2's attention)
                        wo0 = wpool.tile([P, NIT, 512], F32R, tag="wq")
                        wo1 = wpool.tile([P, NIT, 512], F32R, tag="wk")
                        wo_r = wo_d.rearrange("(i p) o -> p i o", p=P)

                        def wo_dma(w, oc):
                            def g():
                                for it in range(NIT):
                                    nc.sync.dma_start(
                                        w[:, it],
                                        wo_r[:, it, oc * 512:(oc + 1) * 512])
                            return g

                        def o_group(b, m, aob, osb, w, oc, cout_idx):
                            def g():
                                ps = ppool.tile([P, 512], F32, tag="proj")
                                for ct in range(NIT):
                                    nc.tensor.matmul(
                                        ps[:], aob[:, ct], w[:, ct],
                                        start=(ct == 0), stop=(ct == NIT - 1))
                                nc.vector.tensor_copy(
                                    osb[:, oc * 512:(oc + 1) * 512], ps[:])
                                if oc == 1:
                                    nc.sync.dma_start(out_d[b, m], osb[:])
                            return g

                        def o_stage(b, m, cout):
                            def g():
                                aob = ob_pool.tile([P, NIT, P], F32R,
                                                   tag="aob")
                                osb = ob_pool.tile([P, D], F32, tag="osb")
                                nc.sync.dma_start(
                                    aob[:],
                                    cout[2 * b:2 * b + 2].rearrange(
                                        "s (c p) t -> p (s c) t", p=P))
                                pending.append(
                                    o_group(b, m, aob, osb, wo0, 0, None))
                                pending.append(
                                    o_group(b, m, aob, osb, wo1, 1, None))
                            return g

                        pending.append(wo_dma(wo0, 0))
                        pending.append(wo_dma(wo1, 1))
                        for b in range(B):
                            pending.append(o_stage(b, 0, a2a_out0))
                    if tc4 + 1 < NQC:
                        qtc = next_qtc
                emit_collective(1)

                # ---- m=1 output projection (tail) ----------------------
                for b in range(B):
                    aob = ob_pool.tile([P, NIT, P], F32R, tag="aob")
                    osb = ob_pool.tile([P, D], F32, tag="osb")
                    nc.sync.dma_start(
                        aob[:],
                        a2a_out1[2 * b:2 * b + 2].rearrange(
                            "s (c p) t -> p (s c) t", p=P))
                    for oc in range(2):
                        w = (wo0, wo1)[oc]
                        ps = ppool.tile([P, 512], F32, tag="proj")
                        for ct in range(NIT):
                            nc.tensor.matmul(
                                ps[:], aob[:, ct], w[:, ct],
                                start=(ct == 0), stop=(ct == NIT - 1))
                        nc.vector.tensor_copy(
                            osb[:, oc * 512:(oc + 1) * 512], ps[:])
                    nc.sync.dma_start(out_d[b, 1], osb[:])

    _split_multiwaits(nc)
    return nc


_NC_CACHE = None


def _get_nc():
    global _NC_CACHE
    if _NC_CACHE is None:
        _NC_CACHE = _build_nc()
    return _NC_CACHE


def make_in_maps(x, W_Q, W_K, W_V, W_O):
    wqt = np.ascontiguousarray(W_Q.T)
    wkt = np.ascontiguousarray(W_K.T)
    wvt = np.ascontiguousarray(W_V.T)
    wot = np.ascontiguousarray(W_O.T)
    ones = np.ones((P, NKT * NHP * 2), np.float32)
    in_maps = []
    for c in range(NCORES):
        b, g = c // 2, c % 2
        in_maps.append({
            "xt": np.ascontiguousarray(x[b].T),
            "wq": np.ascontiguousarray(wqt[:, g * CH:(g + 1) * CH]),
            "wk": np.ascontiguousarray(wkt[:, g * CH:(g + 1) * CH]),
            "wv": np.ascontiguousarray(wvt[:, g * CH:(g + 1) * CH]),
            "wo": wot,
            "ones": ones,
        })
    return in_maps


def assemble(results):
    out = np.empty((B, T, D), np.float32)
    for j in range(NCORES):
        o = results[j]["out"]  # [B, 2, 128, D]
        for b in range(B):
            for m in range(2):
                r0 = m * 1024 + j * P
                out[b, r0:r0 + P, :] = o[b, m]
    return out


def kernel(x, W_Q, W_K, W_V, W_O):
    x = np.asarray(x, np.float32)
    in_maps = make_in_maps(x, np.asarray(W_Q, np.float32),
                           np.asarray(W_K, np.float32),
                           np.asarray(W_V, np.float32),
                           np.asarray(W_O, np.float32))
    nc = _get_nc()
    res = run_bass_kernel_spmd(nc, in_maps, core_ids=list(range(NCORES)))
    return assemble(res.results)



# revision 33
# speedup vs baseline: 1.0057x; 1.0057x over previous
"""Causal multi-head attention (B=4, T=2048, D=1024, H=16) on 8 NeuronCores.

Sharding:
  stage 1 (QKV proj + attention): core c -> batch c%4, head-group c//4
    (8 of 16 heads, 512 of 1024 channels). Data-parallel on B, tensor-
    parallel on heads.
  stage 2 (output projection): FOUR AllToAlls, one per 512-token q-chunk.
    Core j owns rows {qc*512 + j*64 .. +64} for every batch; each collective
    re-shards one finished q-chunk so its output projection overlaps the
    next chunk's attention, and the tail after the last chunk is only one
    small collective + 1/4 of the O-projection.

Attention internals run in bf16 (Q^T, K^T, V, probabilities) which keeps
every matmul at 1 PE-cycle/row regardless of width. PV runs q-major:
out[128q, 65] = pt[128k,128q]^T-block-chain @ (V|ones), using the FULL 128
output partitions (the old k-major [65,512] layout wasted half the PE) and
yielding the softmax denominator per-partition, so normalization is a
single per-partition DVE divide - no PE broadcast matmuls. The [q,ch]->
[ch,q] flip needed by the O-projection happens for free inside the
post-collective read via DMA xbar transposes (dma_start_transpose).

Projections stay fp32r (accuracy); psum->sbuf copies convert to bf16.
The t-chunk loop interleaves projections/O-projections with attention as
PE filler work, with deadline-aware pacing so chunk-3's K/V projections
land inside qc3's ACT-bound window without stalling its diagonal chains.
"""
import numpy as np
import ml_dtypes

import concourse.bass as bass
import concourse.mybir as mybir
import concourse.tile as tile
from concourse.tile import add_dep_helper
from concourse.bass_utils import run_bass_kernel_spmd

F32 = mybir.dt.float32
F32R = mybir.dt.float32r
BF16 = mybir.dt.bfloat16
EXP = mybir.ActivationFunctionType.Exp

P = 128
B, T, D = 4, 2048, 1024
H, HD = 16, 64
NCORES = 8
CH = D // 2          # channels per core (8 heads)
NHP = 4              # head pairs per core
NKT = T // P         # 16 k-tiles
NQC = T // 512       # 4 q-chunks
NIT = D // P         # 8 input-dim tiles


def _split_multiwaits(nc) -> int:
    """walrus here rejects >1 sem wait per instruction; split extras into
    wait-only NoOps on the same engine."""
    nsplit = 0
    for f in nc.m.functions:
        for bb in f.blocks:
            if not any(
                i.sync_info is not None and i.sync_info.on_wait is not None
                and len(i.sync_info.on_wait) > 1 for i in bb.instructions
            ):
                continue
            new_list = []
            for inst in bb.instructions:
                si = inst.sync_info
                if si is not None and si.on_wait is not None and len(si.on_wait) > 1:
                    waits = list(si.on_wait)
                    for k, w in enumerate(waits[:-1]):
                        n = mybir.InstNoOp(
                            name=f"{inst.name}-wsplit{k}", ins=[], outs=[])
                        n.engine = inst.engine
                        n.sync_info = mybir.SyncInfo(on_wait=[w], on_update=[])
                        new_list.append(n)
                        nsplit += 1
                    inst.sync_info = mybir.SyncInfo(
                        on_wait=[waits[-1]], on_update=list(si.on_update or []))
                new_list.append(inst)
            bb.instructions = new_list
    return nsplit


def _build_nc(sim: bool = False, norm_mode: str = "recip",
              dbg: bool = False):
    nc = bass.Bass("TRN2", target_bir_lowering=False, debug=False,
                   num_devices=NCORES)
    xt_d = nc.dram_tensor("xt", [D, T], F32R, kind="ExternalInput").ap()
    wq_d = nc.dram_tensor("wq", [D, CH], F32R, kind="ExternalInput").ap()
    wk_d = nc.dram_tensor("wk", [D, CH], F32R, kind="ExternalInput").ap()
    wv_d = nc.dram_tensor("wv", [D, CH], F32R, kind="ExternalInput").ap()
    wo_d = nc.dram_tensor("wo", [D, D], BF16, kind="ExternalInput").ap()
    out_d = nc.dram_tensor("out", [NQC, 2, P, D], F32,
                           kind="ExternalOutput").ap()
    a2a_in = [nc.dram_tensor(f"a2a_in{m}", [NCORES, 64, CH], BF16).ap()
              for m in range(NQC)]
    a2a_out = [nc.dram_tensor(f"a2a_out{m}", [NCORES, 64, CH], BF16).ap()
               for m in range(NQC)]
    if dbg:  # dbg=1: tensors + copies; dbg=2: tensors only (layout probe)
        dbg_in = [nc.dram_tensor(f"dbg_in{m}", [NCORES, 64, CH], BF16,
                                 kind="ExternalOutput").ap()
                  for m in range(NQC)]
        dbg_out = [nc.dram_tensor(f"dbg_out{m}", [NCORES, 64, CH], BF16,
                                  kind="ExternalOutput").ap()
                   for m in range(NQC)]
    a2a_in_r = [a.rearrange("s q c -> (s q) c") for a in a2a_in]
    a2a_out_r = [a.rearrange("s q c -> (s q) c") for a in a2a_out]

    scale = float(1.0 / np.sqrt(HD))

    with tile.TileContext(nc) as tc:
        with (
            tc.tile_pool(name="persist", bufs=1) as persist,
        ):
            # ---- persistent SBUF tensors -------------------------------
            kt_s = persist.tile([P, NHP, T], BF16)              # K^T
            # V | ones | pad: 66-element slots keep every head-slot 4-byte
            # aligned (65 x 2B = 130B slots corrupt packed bf16 DVE writes
            # on hardware) and give the ones column a private 32-bit word
            va = persist.tile([P, NKT, NHP, 2, HD + 2], BF16)
            pt = persist.tile([P, NKT, 2, 512], BF16)           # probs
            wo16 = persist.tile([P, NIT, D], BF16)              # W_O^T

            with (
                tc.tile_pool(name="wpool", bufs=1) as wpool,
                tc.tile_pool(name="xpool", bufs=2) as xpool,
                tc.tile_pool(name="qpool", bufs=2) as qpool,
                tc.tile_pool(name="mpool", bufs=1) as mpool,
                tc.tile_pool(name="aoq_pool", bufs=8) as aoq_pool,
                tc.tile_pool(name="aob_pool", bufs=16) as aob_pool,
                tc.tile_pool(name="osb_pool", bufs=2) as osb_pool,
                tc.tile_pool(name="rpool", bufs=4) as rpool,
                tc.tile_pool(name="ppool", bufs=2, space="PSUM") as ppool,
                tc.tile_pool(name="ps_s", bufs=2, space="PSUM") as ps_s,
                tc.tile_pool(name="ps_pv", bufs=2, space="PSUM") as ps_pv,
            ):
                wq = wpool.tile([P, NIT, CH], F32R)
                wk = wpool.tile([P, NIT, CH], F32R)
                wv = wpool.tile([P, NIT, CH], F32R)
                xt_r = xt_d.rearrange("(i p) t -> p i t", p=P)
                wo_r = wo_d.rearrange("(i p) o -> p i o", p=P)
                xtc0 = xpool.tile([P, NIT, 512], F32R, tag="xtc")
                for it in range(NIT):
                    nc.sync.dma_start(xtc0[:, it], xt_r[:, it, 0:512])
                    nc.sync.dma_start(wv[:, it], wv_d.rearrange(
                        "(i p) o -> p i o", p=P)[:, it])
                for it in range(NIT):
                    nc.sync.dma_start(wq[:, it], wq_d.rearrange(
                        "(i p) o -> p i o", p=P)[:, it])
                    nc.sync.dma_start(wk[:, it], wk_d.rearrange(
                        "(i p) o -> p i o", p=P)[:, it])

                # ones column of V (softmax denominator source); on DVE so
                # it serializes with the V psum->sbuf copies
                nc.vector.memset(va[:, :, :, :, HD], 1.0)
                masks = []
                for i in range(4):
                    m = mpool.tile([P, 512], BF16, tag=f"mask{i}")
                    nc.gpsimd.memset(m[:], 1.0)
                    nc.gpsimd.affine_select(
                        out=m[:], in_=m[:],
                        compare_op=mybir.AluOpType.is_ge,
                        fill=0.0, base=-P * i, channel_multiplier=-1,
                        pattern=[[1, 512]])
                    masks.append(m)

                # ---- filler queue: [fn, deadline_unit_or_None] ---------
                pending = []
                unit_ctr = [0]
                filler_acc = [0.0]

                def emit_fillers(remaining):
                    u = unit_ctr[0]
                    i = 0
                    while i < len(pending):
                        if pending[i][1] is not None and pending[i][1] <= u:
                            pending.pop(i)[0]()
                        else:
                            i += 1
                    if not pending:
                        return
                    filler_acc[0] += len(pending) / max(1, remaining)
                    while filler_acc[0] >= 1.0 and pending:
                        filler_acc[0] -= 1.0
                        pending.pop(0)[0]()

                def drain_pending():
                    while pending:
                        pending.pop(0)[0]()

                # ---- projections --------------------------------------
                def project(tc4, xtc=None):
                    """Build QKV projection group closures for chunk tc4.
                    Returns (qtc, qgroups, kgroups, vgroups)."""
                    if xtc is None:
                        xtc = xpool.tile([P, NIT, 512], F32R, tag="xtc")
                        for it in range(NIT):
                            nc.sync.dma_start(
                                xtc[:, it],
                                xt_r[:, it, tc4 * 512:(tc4 + 1) * 512])
                    qtc = qpool.tile([P, NHP, 512], BF16, tag="qtc")

                    def qk_group(w, dst, dsl, ot):
                        def g():
                            ps = ppool.tile([P, 512], F32, tag="proj")
                            for it in range(NIT):
                                nc.tensor.matmul(
                                    ps[:], w[:, it, ot * P:(ot + 1) * P],
                                    xtc[:, it], start=(it == 0),
                                    stop=(it == NIT - 1))
                            nc.vector.tensor_copy(dst[:, ot, dsl], ps[:])
                        return g

                    def v_group(tt4):
                        def g():
                            ps = ppool.tile([P, 512], F32, tag="proj")
                            for it in range(NIT):
                                nc.tensor.matmul(
                                    ps[:], xtc[:, it, tt4 * P:(tt4 + 1) * P],
                                    wv[:, it], start=(it == 0),
                                    stop=(it == NIT - 1))
                            nc.vector.tensor_copy(
                                va[:, tc4 * 4 + tt4, :, :, 0:HD],
                                ps[:].rearrange("p (hp h d) -> p hp h d",
                                                hp=NHP, h=2))
                        return g

                    qg = [qk_group(wq, qtc, slice(0, 512), ot)
                          for ot in range(NHP)]
                    kg = [qk_group(wk, kt_s,
                                   slice(tc4 * 512, (tc4 + 1) * 512), ot)
                          for ot in range(NHP)]
                    vg = [v_group(tt4) for tt4 in range(4)]
                    return qtc, qg, kg, vg

                # chunk 0: V first via 4 concurrent psums (it-major) so each
                # arriving (x, wv) DMA chunk feeds 4 matmuls during the
                # DMA-bound startup ramp
                qtc, qg0, kg0, vg0 = project(0, xtc=xtc0)

                def v_block0():
                    pss = [
                        ppool.tile([P, 512], F32, tag="proj", name="v0ps0"),
                        ppool.tile([P, 512], F32, tag="proj", name="v0ps1"),
                        ps_pv.tile([P, 512], F32, tag="pv", name="v0ps2"),
                        ps_pv.tile([P, 512], F32, tag="pv", name="v0ps3"),
                    ]
                    for it in range(NIT):
                        for tt4 in range(4):
                            nc.tensor.matmul(
                                pss[tt4][:],
                                xtc0[:, it, tt4 * P:(tt4 + 1) * P],
                                wv[:, it], start=(it == 0),
                                stop=(it == NIT - 1))
                    for tt4 in range(4):
                        nc.vector.tensor_copy(
                            va[:, tt4, :, :, 0:HD],
                            pss[tt4][:].rearrange(
                                "p (hp h d) -> p hp h d", hp=NHP, h=2))

                v_block0()
                for ot in range(NHP):
                    qg0[ot]()
                    kg0[ot]()

                # ---- collectives / stage 2 -----------------------------
                # Tile does not track DRAM-tensor data flow, so the
                # aoq-DMA -> collective -> transposed-read chain needs
                # explicit dependencies.
                aob_map = {}
                aoq_dmas = {m: [] for m in range(NQC)}
                cc_insts = {}

                def emit_collective(m):
                    if sim:
                        cc = nc.sync.dma_start(a2a_out[m], a2a_in[m])
                    else:
                        cc = nc.gpsimd.collective_compute(
                            "AllToAll", mybir.AluOpType.bypass,
                            replica_groups=[list(range(NCORES))],
                            ins=[a2a_in[m]], outs=[a2a_out[m]])
                    for d in aoq_dmas[m]:
                        add_dep_helper(cc.ins, d.ins,
                                       reason="collective reads a2a_in")
                    cc_insts[m] = cc

                identity = mpool.tile([P, P], BF16)
                nc.gpsimd.memset(identity[:], 1.0)
                nc.gpsimd.affine_select(
                    out=identity[:], in_=identity[:],
                    compare_op=mybir.AluOpType.is_ge,
                    fill=0.0, base=0, channel_multiplier=-1,
                    pattern=[[1, P]])
                nc.gpsimd.affine_select(
                    out=identity[:], in_=identity[:],
                    compare_op=mybir.AluOpType.is_ge,
                    fill=0.0, base=0, channel_multiplier=1,
                    pattern=[[-1, P]])

                def transpose_group(m, g, half):
                    """Plain DMA read of one quarter of the q-major collective
                    output, then 4 PE transposes into ch-major aob tiles (the
                    DMA-xbar transpose read raced on HW)."""
                    def gfn():
                        araw = osb_pool.tile([P, 512], BF16, tag="araw")
                        d = nc.sync.dma_start(
                            araw[:],
                            a2a_out_r[m][g * 256 + half * P:
                                         g * 256 + (half + 1) * P])
                        add_dep_helper(d.ins, cc_insts[m].ins,
                                       reason="aob reads a2a_out")
                        for cs in range(4):
                            tp = ppool.tile([P, 1024], BF16, tag="proj",
                                            name=f"tp{m}_{g}_{half}_{cs}")
                            nc.tensor.matmul(
                                tp[:, 0:P], araw[:, cs * P:(cs + 1) * P],
                                identity[:], start=True, stop=True,
                                is_transpose=True)
                            nc.vector.tensor_copy(
                                aob_map[m][g * 4 + cs][:, half * P:
                                                       (half + 1) * P],
                                tp[:, 0:P])
                    return gfn

                def queue_transposes(m, deadlines=(2, 5, 8, 11)):
                    aob_map[m] = [aob_pool.tile([P, 256], BF16, tag="aob",
                                                name=f"aob{m}_{ct}")
                                  for ct in range(NIT)]
                    k = 0
                    for g in range(2):
                        for half in range(2):
                            pending.append([transpose_group(m, g, half),
                                            deadlines[k]])
                            k += 1

                wo_dmas = []

                def o_group(m, tt, oc, osb):
                    def g():
                        ps = ppool.tile([P, 512], F32, tag="proj")
                        for ct in range(NIT):
                            mm = nc.tensor.matmul(
                                ps[:], aob_map[m][ct][:, tt * P:(tt + 1) * P],
                                wo16[:, ct, oc * 512:(oc + 1) * 512],
                                start=(ct == 0), stop=(ct == NIT - 1))
                            # Tile misses the RAW dep on the moving operand
                            # for some DMA-written tiles; make it explicit.
                            add_dep_helper(mm.ins, wo_dmas[ct].ins,
                                           reason="oproj reads wo16")
                        nc.vector.tensor_copy(
                            osb[:, oc * 512:(oc + 1) * 512], ps[:])
                        if oc == 1:
                            nc.sync.dma_start(out_d[m, tt], osb[:])
                    return g

                def queue_oproj(m):
                    for tt in range(2):
                        osb = osb_pool.tile([P, D], F32, tag="osb")
                        pending.append([o_group(m, tt, 0, osb), None])
                        pending.append([o_group(m, tt, 1, osb), None])

                def wo_unit():
                    for it in range(NIT):
                        wo_dmas.append(
                            nc.sync.dma_start(wo16[:, it], wo_r[:, it]))

                # ---- attention -----------------------------------------
                def attend(hp, qc, qtc, aoq_tiles):
                    nkt = 4 * (qc + 1)
                    s2s = {}

                    def qk(kt):
                        ksl = slice(kt * P, (kt + 1) * P)
                        f0 = max(0, kt - 4 * qc) * P
                        s2 = ps_s.tile([P, 1024], F32, tag="s2")
                        nc.tensor.matmul(s2[:, f0:512], kt_s[0:64, hp, ksl],
                                         qtc[0:64, hp, f0:],
                                         start=True, stop=True)
                        nc.tensor.matmul(s2[:, 512 + f0:1024],
                                         kt_s[64:128, hp, ksl],
                                         qtc[64:128, hp, f0:],
                                         start=True, stop=True)
                        s2s[kt] = s2

                    def soft(kt):
                        s2 = s2s.pop(kt)
                        di = kt - 4 * qc
                        f0 = max(0, di) * P
                        ptv = pt[:, kt]
                        if f0 > 0:
                            s2v = s2[:].rearrange("p (a b) -> p a b", a=2)
                            nc.scalar.activation(
                                ptv[:, :, f0:], s2v[:, :, f0:], EXP,
                                scale=scale)
                        else:
                            nc.scalar.activation(
                                ptv.rearrange("p a b -> p (a b)"), s2[:],
                                EXP, scale=scale)
                        if di >= 0:
                            nc.vector.tensor_mul(
                                ptv[:, :, f0:], ptv[:, :, f0:],
                                masks[di][:, None, f0:].to_broadcast(
                                    (P, 2, 512 - f0)))
                            qb = di
                            for h in range(2):
                                pv = ps_pv.tile([P, 512], F32, tag="pv")
                                for kt2 in range(kt + 1):
                                    nc.tensor.matmul(
                                        pv[:, 0:65],
                                        pt[:, kt2, h, qb * P:(qb + 1) * P],
                                        va[:, kt2, hp, h, 0:HD + 1],
                                        start=(kt2 == 0), stop=(kt2 == kt))
                                dst = aoq_tiles[qb][:, hp, h]
                                if norm_mode == "div":
                                    nc.vector.tensor_tensor(
                                        dst, pv[:, 0:64],
                                        pv[:, 64:65].to_broadcast((P, 64)),
                                        op=mybir.AluOpType.divide)
                                else:
                                    rden = rpool.tile([P, 1], F32, tag="rd")
                                    nc.vector.reciprocal(rden[:], pv[:, 64:65])
                                    nc.vector.tensor_mul(
                                        dst, pv[:, 0:64],
                                        rden[:].to_broadcast((P, 64)))
                            if hp == NHP - 1:
                                d = nc.sync.dma_start(
                                    a2a_in_r[qc][qb * P:(qb + 1) * P],
                                    aoq_tiles[qb][:])
                                aoq_dmas[qc].append(d)
                        unit_ctr[0] += 1
                        emit_fillers((nkt - 1 - kt) + (NHP - 1 - hp) * nkt)

                    qk(0)
                    for kt in range(1, nkt):
                        qk(kt)
                        soft(kt - 1)
                    soft(nkt - 1)

                # ---- main interleaved loop -----------------------------
                nxt = None
                for tc4 in range(NQC):
                    unit_ctr[0] = 0
                    nkt = 4 * (tc4 + 1)
                    if tc4 == 0:
                        pending.append([wo_unit, None])
                        nxt = project(1)
                        for ot in range(NHP):
                            pending.append([nxt[1][ot], None])   # Q
                            pending.append([nxt[2][ot], None])   # K
                        for tt4 in range(4):
                            pending.append([nxt[3][tt4], None])  # V
                    elif tc4 == 1:
                        queue_transposes(0)
                        nxt = project(2)
                        for ot in range(NHP):
                            pending.append([nxt[1][ot], None])
                            pending.append([nxt[2][ot], None])
                        for tt4 in range(4):
                            pending.append([nxt[3][tt4], None])
                        queue_oproj(0)
                    elif tc4 == 2:
                        queue_transposes(1)
                        nxt = project(3)
                        for ot in range(NHP):
                            pending.append([nxt[1][ot], None])   # Q only
                        queue_oproj(1)
                    elif tc4 == 3:
                        # K/V of chunk 3 land inside qc3's window, deadline'd
                        # ahead of the attention instructions that read them
                        # (PE is in-order: a chain emitted before its V-group
                        # would deadlock).
                        queue_transposes(2)
                        pending.append([nxt[2][0], 8])            # K0
                        for tt4 in range(4):
                            pending.append([nxt[3][tt4], 9 + tt4])  # V
                        for ot in range(1, NHP):
                            pending.append([nxt[2][ot], ot * 16 + 8])
                        queue_oproj(2)

                    aoq_tiles = [aoq_pool.tile([P, NHP, 2, HD], BF16,
                                               tag="aoq", name=f"aoq{tc4}_{i}")
                                 for i in range(4)]
                    for hp in range(NHP):
                        attend(hp, tc4, qtc, aoq_tiles)

                    # non-deadline stragglers must not leak past the window
                    # (next chunk's attention reads their outputs)
                    i = 0
                    while i < len(pending):
                        if pending[i][1] is None:
                            pending.pop(i)[0]()
                        else:
                            i += 1

                    emit_collective(tc4)
                    if dbg == 1:
                        d1 = nc.sync.dma_start(dbg_in[tc4], a2a_in[tc4])
                        d2 = nc.sync.dma_start(dbg_out[tc4], a2a_out[tc4])
                        add_dep_helper(d1.ins, cc_insts[tc4].ins, reason="dbg")
                        add_dep_helper(d2.ins, cc_insts[tc4].ins, reason="dbg")
                    if tc4 < 3:
                        qtc = nxt[0]

                # ---- tail: last O-projection ---------------------------
                drain_pending()
                queue_transposes(3)
                queue_oproj(3)
                drain_pending()
                if dbg == 5:
                    dump0 = nc.dram_tensor(
                        "dump0", [NCORES, 64, CH], BF16,
                        kind="ExternalOutput").ap()
                    nc.sync.dma_start(dump0, a2a_in[0])
                if dbg == 6:
                    dump0 = nc.dram_tensor(
                        "dump0", [512, 256], F32R,
                        kind="ExternalOutput").ap()
                    nc.sync.dma_start(dump0, xt_d[0:512, 0:256])
                if dbg == 4:
                    vadump = nc.dram_tensor(
                        "vadump", [P, NKT, NHP, 2, HD + 2], BF16,
                        kind="ExternalOutput").ap()
                    ptdump = nc.dram_tensor(
                        "ptdump", [P, NKT, 2, 512], BF16,
                        kind="ExternalOutput").ap()
                    a2adump = nc.dram_tensor(
                        "a2adump", [NCORES, 64, CH], BF16,
                        kind="ExternalOutput").ap()
                    nc.sync.dma_start(vadump, va[:])
                    nc.sync.dma_start(ptdump, pt[:])
                    nc.sync.dma_start(a2adump, a2a_in[0])
                if dbg == 3:
                    # end-of-kernel snapshots: after all consumers, so they
                    # cannot mask ordering bugs
                    for m in range(NQC):
                        d1 = nc.sync.dma_start(dbg_in[m], a2a_in[m])
                        d2 = nc.sync.dma_start(dbg_out[m], a2a_out[m])
                        add_dep_helper(d1.ins, cc_insts[m].ins, reason="dbg")
                        add_dep_helper(d2.ins, cc_insts[m].ins, reason="dbg")

    _split_multiwaits(nc)
    return nc


_NC_CACHE = None


def _get_nc():
    global _NC_CACHE
    if _NC_CACHE is None:
        _NC_CACHE = _build_nc()
    return _NC_CACHE


def make_in_maps(x, W_Q, W_K, W_V, W_O):
    wqt = np.ascontiguousarray(W_Q.T)
    wkt = np.ascontiguousarray(W_K.T)
    wvt = np.ascontiguousarray(W_V.T)
    wot = np.ascontiguousarray(W_O.T).astype(ml_dtypes.bfloat16)
    in_maps = []
    for c in range(NCORES):
        g, b = c // 4, c % 4
        in_maps.append({
            "xt": np.ascontiguousarray(x[b].T),
            "wq": np.ascontiguousarray(wqt[:, g * CH:(g + 1) * CH]),
            "wk": np.ascontiguousarray(wkt[:, g * CH:(g + 1) * CH]),
            "wv": np.ascontiguousarray(wvt[:, g * CH:(g + 1) * CH]),
            "wo": wot,
        })
    return in_maps


def assemble(results):
    out = np.empty((B, T, D), np.float32)
    for j in range(NCORES):
        o = results[j]["out"]  # [NQC, 2, 128, D]
        for qc in range(NQC):
            for b in range(B):
                r0 = qc * 512 + j * 64
                out[b, r0:r0 + 64, :] = o[qc, b // 2,
                                          (b % 2) * 64:(b % 2) * 64 + 64, :]
    return out


def kernel(x, W_Q, W_K, W_V, W_O):
    x = np.asarray(x, np.float32)
    in_maps = make_in_maps(x, np.asarray(W_Q, np.float32),
                           np.asarray(W_K, np.float32),
                           np.asarray(W_V, np.float32),
                           np.asarray(W_O, np.float32))
    nc = _get_nc()
    res = run_bass_kernel_spmd(nc, in_maps, core_ids=list(range(NCORES)))
    return assemble(res.results)


# revision 38
# speedup vs baseline: 1.0140x; 1.0082x over previous
"""Causal multi-head attention (B=4, T=2048, D=1024, H=16) on 8 NeuronCores.

Sharding:
  stage 1 (QKV proj + attention): core c -> batch c%4, head-group c//4
    (8 of 16 heads, 512 of 1024 channels). Data-parallel on B, tensor-
    parallel on heads.
  stage 2 (output projection): FOUR AllToAlls, one per 512-token q-chunk.
    Core j owns rows {qc*512 + j*64 .. +64} for every batch; each collective
    re-shards one finished q-chunk so its output projection overlaps the
    next chunk's attention, and the tail after the last chunk is only one
    small collective + 1/4 of the O-projection.

Attention internals run in bf16 (Q^T, K^T, V, probabilities) which keeps
every matmul at 1 PE-cycle/row regardless of width. PV runs q-major:
out[128q, 65] = pt[128k,128q]^T-block-chain @ (V|ones), using the FULL 128
output partitions (the old k-major [65,512] layout wasted half the PE) and
yielding the softmax denominator per-partition, so normalization is a
single per-partition DVE divide - no PE broadcast matmuls. The [q,ch]->
[ch,q] flip needed by the O-projection happens for free inside the
post-collective read via DMA xbar transposes (dma_start_transpose).

Projections stay fp32r (accuracy); psum->sbuf copies convert to bf16.
The t-chunk loop interleaves projections/O-projections with attention as
PE filler work, with deadline-aware pacing so chunk-3's K/V projections
land inside qc3's ACT-bound window without stalling its diagonal chains.
"""
import numpy as np
import ml_dtypes

import concourse.bass as bass
import concourse.mybir as mybir
import concourse.tile as tile
from concourse.tile import add_dep_helper
from concourse.bass_utils import run_bass_kernel_spmd

F32 = mybir.dt.float32
F32R = mybir.dt.float32r
BF16 = mybir.dt.bfloat16
EXP = mybir.ActivationFunctionType.Exp

P = 128
B, T, D = 4, 2048, 1024
H, HD = 16, 64
NCORES = 8
CH = D // 2          # channels per core (8 heads)
NHP = 4              # head pairs per core
NKT = T // P         # 16 k-tiles
NQC = T // 512       # 4 q-chunks
NIT = D // P         # 8 input-dim tiles


def _split_multiwaits(nc) -> int:
    """walrus here rejects >1 sem wait per instruction; split extras into
    wait-only NoOps on the same engine."""
    nsplit = 0
    for f in nc.m.functions:
        for bb in f.blocks:
            if not any(
                i.sync_info is not None and i.sync_info.on_wait is not None
                and len(i.sync_info.on_wait) > 1 for i in bb.instructions
            ):
                continue
            new_list = []
            for inst in bb.instructions:
                si = inst.sync_info
                if si is not None and si.on_wait is not None and len(si.on_wait) > 1:
                    waits = list(si.on_wait)
                    for k, w in enumerate(waits[:-1]):
                        n = mybir.InstNoOp(
                            name=f"{inst.name}-wsplit{k}", ins=[], outs=[])
                        n.engine = inst.engine
                        n.sync_info = mybir.SyncInfo(on_wait=[w], on_update=[])
                        new_list.append(n)
                        nsplit += 1
                    inst.sync_info = mybir.SyncInfo(
                        on_wait=[waits[-1]], on_update=list(si.on_update or []))
                new_list.append(inst)
            bb.instructions = new_list
    return nsplit


def _build_nc(sim: bool = False, norm_mode: str = "recip",
              dbg: bool = False):
    nc = bass.Bass("TRN2", target_bir_lowering=False, debug=False,
                   num_devices=NCORES)
    xt_d = nc.dram_tensor("xt", [D, T], F32R, kind="ExternalInput").ap()
    wq_d = nc.dram_tensor("wq", [D, CH], F32R, kind="ExternalInput").ap()
    wk_d = nc.dram_tensor("wk", [D, CH], F32R, kind="ExternalInput").ap()
    wv_d = nc.dram_tensor("wv", [D, CH], F32R, kind="ExternalInput").ap()
    wo_d = nc.dram_tensor("wo", [D, D], BF16, kind="ExternalInput").ap()
    out_d = nc.dram_tensor("out", [NQC, 2, P, D], F32,
                           kind="ExternalOutput").ap()
    a2a_in = [nc.dram_tensor(f"a2a_in{m}", [NCORES, 64, CH], BF16).ap()
              for m in range(NQC)]
    a2a_out = [nc.dram_tensor(f"a2a_out{m}", [NCORES, 64, CH], BF16).ap()
               for m in range(NQC)]
    if dbg:  # dbg=1: tensors + copies; dbg=2: tensors only (layout probe)
        dbg_in = [nc.dram_tensor(f"dbg_in{m}", [NCORES, 64, CH], BF16,
                                 kind="ExternalOutput").ap()
                  for m in range(NQC)]
        dbg_out = [nc.dram_tensor(f"dbg_out{m}", [NCORES, 64, CH], BF16,
                                  kind="ExternalOutput").ap()
                   for m in range(NQC)]
    a2a_in_r = [a.rearrange("s q c -> (s q) c") for a in a2a_in]
    a2a_out_r = [a.rearrange("s q c -> (s q) c") for a in a2a_out]

    scale = float(1.0 / np.sqrt(HD))

    with tile.TileContext(nc) as tc:
        with (
            tc.tile_pool(name="persist", bufs=1) as persist,
        ):
            # ---- persistent SBUF tensors -------------------------------
            kt_s = persist.tile([P, NHP, T], BF16)              # K^T
            # V | ones | pad: 66-element slots keep every head-slot 4-byte
            # aligned (65 x 2B = 130B slots corrupt packed bf16 DVE writes
            # on hardware) and give the ones column a private 32-bit word
            va = persist.tile([P, NKT, NHP, 2, HD + 2], BF16)
            pt = persist.tile([P, NKT, 2, 512], BF16)           # probs
            wo16 = persist.tile([P, NIT, D], BF16)              # W_O^T

            with (
                tc.tile_pool(name="wpool", bufs=1) as wpool,
                tc.tile_pool(name="xpool", bufs=2) as xpool,
                tc.tile_pool(name="qpool", bufs=2) as qpool,
                tc.tile_pool(name="mpool", bufs=1) as mpool,
                tc.tile_pool(name="aoq_pool", bufs=8) as aoq_pool,
                tc.tile_pool(name="aob_pool", bufs=16) as aob_pool,
                tc.tile_pool(name="osb_pool", bufs=2) as osb_pool,
                tc.tile_pool(name="rpool", bufs=4) as rpool,
                tc.tile_pool(name="ppool", bufs=2, space="PSUM") as ppool,
                tc.tile_pool(name="ps_s", bufs=2, space="PSUM") as ps_s,
                tc.tile_pool(name="ps_pv", bufs=2, space="PSUM") as ps_pv,
            ):
                wq = wpool.tile([P, NIT, CH], F32R)
                wk = wpool.tile([P, NIT, CH], F32R)
                wv = wpool.tile([P, NIT, CH], F32R)
                xt_r = xt_d.rearrange("(i p) t -> p i t", p=P)
                wo_r = wo_d.rearrange("(i p) o -> p i o", p=P)
                xtc0 = xpool.tile([P, NIT, 512], F32R, tag="xtc")
                for it in range(NIT):
                    nc.sync.dma_start(xtc0[:, it], xt_r[:, it, 0:512])
                    nc.sync.dma_start(wv[:, it], wv_d.rearrange(
                        "(i p) o -> p i o", p=P)[:, it])
                for it in range(NIT):
                    nc.sync.dma_start(wq[:, it], wq_d.rearrange(
                        "(i p) o -> p i o", p=P)[:, it])
                    nc.sync.dma_start(wk[:, it], wk_d.rearrange(
                        "(i p) o -> p i o", p=P)[:, it])

                # ones column of V (softmax denominator source); on DVE so
                # it serializes with the V psum->sbuf copies
                nc.vector.memset(va[:, :, :, :, HD], 1.0)
                masks = []
                for i in range(4):
                    m = mpool.tile([P, 512], BF16, tag=f"mask{i}")
                    nc.gpsimd.memset(m[:], 1.0)
                    nc.gpsimd.affine_select(
                        out=m[:], in_=m[:],
                        compare_op=mybir.AluOpType.is_ge,
                        fill=0.0, base=-P * i, channel_multiplier=-1,
                        pattern=[[1, 512]])
                    masks.append(m)

                # ---- filler queue: [fn, deadline_unit_or_None] ---------
                pending = []
                unit_ctr = [0]
                filler_acc = [0.0]

                def emit_fillers(remaining):
                    u = unit_ctr[0]
                    i = 0
                    while i < len(pending):
                        if pending[i][1] is not None and pending[i][1] <= u:
                            pending.pop(i)[0]()
                        else:
                            i += 1
                    if not pending:
                        return
                    filler_acc[0] += len(pending) / max(1, remaining)
                    while filler_acc[0] >= 1.0 and pending:
                        filler_acc[0] -= 1.0
                        pending.pop(0)[0]()

                def drain_pending():
                    while pending:
                        pending.pop(0)[0]()

                # ---- projections --------------------------------------
                def project(tc4, xtc=None):
                    """Build QKV projection group closures for chunk tc4.
                    Returns (qtc, qgroups, kgroups, vgroups)."""
                    if xtc is None:
                        xtc = xpool.tile([P, NIT, 512], F32R, tag="xtc")
                        for it in range(NIT):
                            nc.sync.dma_start(
                                xtc[:, it],
                                xt_r[:, it, tc4 * 512:(tc4 + 1) * 512])
                    qtc = qpool.tile([P, NHP, 512], BF16, tag="qtc")

                    def qk_group(w, dst, dsl, ot):
                        def g():
                            ps = ppool.tile([P, 512], F32, tag="proj")
                            for it in range(NIT):
                                nc.tensor.matmul(
                                    ps[:], w[:, it, ot * P:(ot + 1) * P],
                                    xtc[:, it], start=(it == 0),
                                    stop=(it == NIT - 1))
                            nc.vector.tensor_copy(dst[:, ot, dsl], ps[:])
                        return g

                    def v_group(tt4):
                        def g():
                            ps = ppool.tile([P, 512], F32, tag="proj")
                            for it in range(NIT):
                                nc.tensor.matmul(
                                    ps[:], xtc[:, it, tt4 * P:(tt4 + 1) * P],
                                    wv[:, it], start=(it == 0),
                                    stop=(it == NIT - 1))
                            nc.vector.tensor_copy(
                                va[:, tc4 * 4 + tt4, :, :, 0:HD],
                                ps[:].rearrange("p (hp h d) -> p hp h d",
                                                hp=NHP, h=2))
                        return g

                    qg = [qk_group(wq, qtc, slice(0, 512), ot)
                          for ot in range(NHP)]
                    kg = [qk_group(wk, kt_s,
                                   slice(tc4 * 512, (tc4 + 1) * 512), ot)
                          for ot in range(NHP)]
                    vg = [v_group(tt4) for tt4 in range(4)]
                    return qtc, qg, kg, vg

                # chunk 0: V first via 4 concurrent psums (it-major) so each
                # arriving (x, wv) DMA chunk feeds 4 matmuls during the
                # DMA-bound startup ramp
                qtc, qg0, kg0, vg0 = project(0, xtc=xtc0)

                def v_block0():
                    pss = [
                        ppool.tile([P, 512], F32, tag="proj", name="v0ps0"),
                        ppool.tile([P, 512], F32, tag="proj", name="v0ps1"),
                        ps_pv.tile([P, 512], F32, tag="pv", name="v0ps2"),
                        ps_pv.tile([P, 512], F32, tag="pv", name="v0ps3"),
                    ]
                    for it in range(NIT):
                        for tt4 in range(4):
                            nc.tensor.matmul(
                                pss[tt4][:],
                                xtc0[:, it, tt4 * P:(tt4 + 1) * P],
                                wv[:, it], start=(it == 0),
                                stop=(it == NIT - 1))
                    for tt4 in range(4):
                        nc.vector.tensor_copy(
                            va[:, tt4, :, :, 0:HD],
                            pss[tt4][:].rearrange(
                                "p (hp h d) -> p hp h d", hp=NHP, h=2))

                v_block0()
                for ot in range(NHP):
                    qg0[ot]()
                    kg0[ot]()

                # ---- collectives / stage 2 -----------------------------
                # Tile does not track DRAM-tensor data flow, so the
                # aoq-DMA -> collective -> transposed-read chain needs
                # explicit dependencies.
                aob_map = {}
                aoq_dmas = {m: [] for m in range(NQC)}
                cc_insts = {}

                def emit_collective(m):
                    if sim:
                        cc = nc.sync.dma_start(a2a_out[m], a2a_in[m])
                    else:
                        cc = nc.gpsimd.collective_compute(
                            "AllToAll", mybir.AluOpType.bypass,
                            replica_groups=[list(range(NCORES))],
                            ins=[a2a_in[m]], outs=[a2a_out[m]])
                    for d in aoq_dmas[m]:
                        add_dep_helper(cc.ins, d.ins,
                                       reason="collective reads a2a_in")
                    cc_insts[m] = cc

                identity = mpool.tile([P, P], BF16)
                nc.gpsimd.memset(identity[:], 1.0)
                nc.gpsimd.affine_select(
                    out=identity[:], in_=identity[:],
                    compare_op=mybir.AluOpType.is_ge,
                    fill=0.0, base=0, channel_multiplier=-1,
                    pattern=[[1, P]])
                nc.gpsimd.affine_select(
                    out=identity[:], in_=identity[:],
                    compare_op=mybir.AluOpType.is_ge,
                    fill=0.0, base=0, channel_multiplier=1,
                    pattern=[[-1, P]])

                def transpose_group(m, g, half):
                    """Plain DMA read of one quarter of the q-major collective
                    output, then 4 PE transposes into ch-major aob tiles (the
                    DMA-xbar transpose read raced on HW)."""
                    def gfn():
                        araw = osb_pool.tile([P, 512], BF16, tag="araw")
                        d = nc.sync.dma_start(
                            araw[:],
                            a2a_out_r[m][g * 256 + half * P:
                                         g * 256 + (half + 1) * P])
                        add_dep_helper(d.ins, cc_insts[m].ins,
                                       reason="aob reads a2a_out")
                        for cs in range(4):
                            tp = ppool.tile([P, 1024], BF16, tag="proj",
                                            name=f"tp{m}_{g}_{half}_{cs}")
                            nc.tensor.matmul(
                                tp[:, 0:P], araw[:, cs * P:(cs + 1) * P],
                                identity[:], start=True, stop=True,
                                is_transpose=True)
                            nc.vector.tensor_copy(
                                aob_map[m][g * 4 + cs][:, half * P:
                                                       (half + 1) * P],
                                tp[:, 0:P])
                    return gfn

                def queue_transposes(m, deadlines=(3, 7, 11, 15)):
                    aob_map[m] = [aob_pool.tile([P, 256], BF16, tag="aob",
                                                name=f"aob{m}_{ct}")
                                  for ct in range(NIT)]
                    k = 0
                    for g in range(2):
                        for half in range(2):
                            pending.append([transpose_group(m, g, half),
                                            deadlines[k]])
                            k += 1

                wo_dmas = []

                def o_group(m, tt, oc, osb):
                    def g():
                        ps = ppool.tile([P, 512], F32, tag="proj")
                        for ct in range(NIT):
                            mm = nc.tensor.matmul(
                                ps[:], aob_map[m][ct][:, tt * P:(tt + 1) * P],
                                wo16[:, ct, oc * 512:(oc + 1) * 512],
                                start=(ct == 0), stop=(ct == NIT - 1))
                            # Tile misses the RAW dep on the moving operand
                            # for some DMA-written tiles; make it explicit.
                            add_dep_helper(mm.ins, wo_dmas[ct].ins,
                                           reason="oproj reads wo16")
                        nc.vector.tensor_copy(
                            osb[:, oc * 512:(oc + 1) * 512], ps[:])
                        if oc == 1:
                            nc.sync.dma_start(out_d[m, tt], osb[:])
                    return g

                def queue_oproj(m):
                    for tt in range(2):
                        osb = osb_pool.tile([P, D], F32, tag="osb")
                        pending.append([o_group(m, tt, 0, osb), None])
                        pending.append([o_group(m, tt, 1, osb), None])

                def wo_unit():
                    for it in range(NIT):
                        wo_dmas.append(
                            nc.sync.dma_start(wo16[:, it], wo_r[:, it]))

                # ---- attention -----------------------------------------
                def attend(hp, qc, qtc, aoq_tiles):
                    nkt = 4 * (qc + 1)
                    s2s = {}

                    def qk(kt):
                        ksl = slice(kt * P, (kt + 1) * P)
                        f0 = max(0, kt - 4 * qc) * P
                        s2 = ps_s.tile([P, 1024], F32, tag="s2")
                        nc.tensor.matmul(s2[:, f0:512], kt_s[0:64, hp, ksl],
                                         qtc[0:64, hp, f0:],
                                         start=True, stop=True)
                        nc.tensor.matmul(s2[:, 512 + f0:1024],
                                         kt_s[64:128, hp, ksl],
                                         qtc[64:128, hp, f0:],
                                         start=True, stop=True)
                        s2s[kt] = s2

                    def soft(kt):
                        s2 = s2s.pop(kt)
                        di = kt - 4 * qc
                        f0 = max(0, di) * P
                        ptv = pt[:, kt]
                        if f0 > 0:
                            s2v = s2[:].rearrange("p (a b) -> p a b", a=2)
                            nc.scalar.activation(
                                ptv[:, :, f0:], s2v[:, :, f0:], EXP,
                                scale=scale)
                        else:
                            nc.scalar.activation(
                                ptv.rearrange("p a b -> p (a b)"), s2[:],
                                EXP, scale=scale)
                        if di >= 0:
                            nc.vector.tensor_mul(
                                ptv[:, :, f0:], ptv[:, :, f0:],
                                masks[di][:, None, f0:].to_broadcast(
                                    (P, 2, 512 - f0)))
                            qb = di
                            for h in range(2):
                                pv = ps_pv.tile([P, 512], F32, tag="pv")
                                for kt2 in range(kt + 1):
                                    nc.tensor.matmul(
                                        pv[:, 0:65],
                                        pt[:, kt2, h, qb * P:(qb + 1) * P],
                                        va[:, kt2, hp, h, 0:HD + 1],
                                        start=(kt2 == 0), stop=(kt2 == kt))
                                dst = aoq_tiles[qb][:, hp, h]
                                if norm_mode == "div":
                                    nc.vector.tensor_tensor(
                                        dst, pv[:, 0:64],
                                        pv[:, 64:65].to_broadcast((P, 64)),
                                        op=mybir.AluOpType.divide)
                                else:
                                    rden = rpool.tile([P, 1], F32, tag="rd")
                                    nc.vector.reciprocal(rden[:], pv[:, 64:65])
                                    nc.vector.tensor_mul(
                                        dst, pv[:, 0:64],
                                        rden[:].to_broadcast((P, 64)))
                            if hp == NHP - 1:
                                d = nc.sync.dma_start(
                                    a2a_in_r[qc][qb * P:(qb + 1) * P],
                                    aoq_tiles[qb][:])
                                aoq_dmas[qc].append(d)
                        unit_ctr[0] += 1
                        emit_fillers((nkt - 1 - kt) + (NHP - 1 - hp) * nkt)

                    qk(0)
                    for kt in range(1, nkt):
                        qk(kt)
                        soft(kt - 1)
                    soft(nkt - 1)

                # ---- main interleaved loop -----------------------------
                nxt = None
                for tc4 in range(NQC):
                    unit_ctr[0] = 0
                    nkt = 4 * (tc4 + 1)
                    if tc4 == 0:
                        pending.append([wo_unit, None])
                        nxt = project(1)
                        for ot in range(NHP):
                            pending.append([nxt[1][ot], None])   # Q
                            pending.append([nxt[2][ot], None])   # K
                        for tt4 in range(4):
                            pending.append([nxt[3][tt4], None])  # V
                    elif tc4 == 1:
                        queue_transposes(0)
                        nxt = project(2)
                        for ot in range(NHP):
                            pending.append([nxt[1][ot], None])
                            pending.append([nxt[2][ot], None])
                        for tt4 in range(4):
                            pending.append([nxt[3][tt4], None])
                        queue_oproj(0)
                    elif tc4 == 2:
                        queue_transposes(1)
                        nxt = project(3)
                        for ot in range(NHP):
                            pending.append([nxt[1][ot], None])   # Q only
                        queue_oproj(1)
                    elif tc4 == 3:
                        # K/V of chunk 3 land inside qc3's window, deadline'd
                        # ahead of the attention instructions that read them
                        # (PE is in-order: a chain emitted before its V-group
                        # would deadlock).
                        queue_transposes(2)
                        pending.append([nxt[2][0], 8])            # K0
                        for tt4 in range(4):
                            pending.append([nxt[3][tt4], 9 + tt4])  # V
                        for ot in range(1, NHP):
                            pending.append([nxt[2][ot], ot * 16 + 8])
                        queue_oproj(2)

                    aoq_tiles = [aoq_pool.tile([P, NHP, 2, HD], BF16,
                                               tag="aoq", name=f"aoq{tc4}_{i}")
                                 for i in range(4)]
                    for hp in range(NHP):
                        attend(hp, tc4, qtc, aoq_tiles)

                    # non-deadline stragglers must not leak past the window
                    # (next chunk's attention reads their outputs)
                    i = 0
                    while i < len(pending):
                        if pending[i][1] is None:
                            pending.pop(i)[0]()
                        else:
                            i += 1

                    emit_collective(tc4)
                    if dbg == 1:
                        d1 = nc.sync.dma_start(dbg_in[tc4], a2a_in[tc4])
                        d2 = nc.sync.dma_start(dbg_out[tc4], a2a_out[tc4])
                        add_dep_helper(d1.ins, cc_insts[tc4].ins, reason="dbg")
                        add_dep_helper(d2.ins, cc_insts[tc4].ins, reason="dbg")
                    if tc4 < 3:
                        qtc = nxt[0]

                # ---- tail: last O-projection ---------------------------
                drain_pending()
                queue_transposes(3, deadlines=(0, 0, 0, 0))
                queue_oproj(3)
                drain_pending()
                if dbg == 5:
                    dump0 = nc.dram_tensor(
                        "dump0", [NCORES, 64, CH], BF16,
                        kind="ExternalOutput").ap()
                    nc.sync.dma_start(dump0, a2a_in[0])
                if dbg == 6:
                    dump0 = nc.dram_tensor(
                        "dump0", [512, 256], F32R,
                        kind="ExternalOutput").ap()
                    nc.sync.dma_start(dump0, xt_d[0:512, 0:256])
                if dbg == 4:
                    vadump = nc.dram_tensor(
                        "vadump", [P, NKT, NHP, 2, HD + 2], BF16,
                        kind="ExternalOutput").ap()
                    ptdump = nc.dram_tensor(
                        "ptdump", [P, NKT, 2, 512], BF16,
                        kind="ExternalOutput").ap()
                    a2adump = nc.dram_tensor(
                        "a2adump", [NCORES, 64, CH], BF16,
                        kind="ExternalOutput").ap()
                    nc.sync.dma_start(vadump, va[:])
                    nc.sync.dma_start(ptdump, pt[:])
                    nc.sync.dma_start(a2adump, a2a_in[0])
                if dbg == 3:
                    # end-of-kernel snapshots: after all consumers, so they
                    # cannot mask ordering bugs
                    for m in range(NQC):
                        d1 = nc.sync.dma_start(dbg_in[m], a2a_in[m])
                        d2 = nc.sync.dma_start(dbg_out[m], a2a_out[m])
                        add_dep_helper(d1.ins, cc_insts[m].ins, reason="dbg")
                        add_dep_helper(d2.ins, cc_insts[m].ins, reason="dbg")

    _split_multiwaits(nc)
    return nc


_NC_CACHE = None


def _get_nc():
    global _NC_CACHE
    if _NC_CACHE is None:
        _NC_CACHE = _build_nc()
    return _NC_CACHE


def make_in_maps(x, W_Q, W_K, W_V, W_O):
    wqt = np.ascontiguousarray(W_Q.T)
    wkt = np.ascontiguousarray(W_K.T)
    wvt = np.ascontiguousarray(W_V.T)
    wot = np.ascontiguousarray(W_O.T).astype(ml_dtypes.bfloat16)
    in_maps = []
    for c in range(NCORES):
        g, b = c // 4, c % 4
        in_maps.append({
            "xt": np.ascontiguousarray(x[b].T),
            "wq": np.ascontiguousarray(wqt[:, g * CH:(g + 1) * CH]),
            "wk": np.ascontiguousarray(wkt[:, g * CH:(g + 1) * CH]),
            "wv": np.ascontiguousarray(wvt[:, g * CH:(g + 1) * CH]),
            "wo": wot,
        })
    return in_maps


def assemble(results):
    out = np.empty((B, T, D), np.float32)
    for j in range(NCORES):
        o = results[j]["out"]  # [NQC, 2, 128, D]
        for qc in range(NQC):
            for b in range(B):
                r0 = qc * 512 + j * 64
                out[b, r0:r0 + 64, :] = o[qc, b // 2,
                                          (b % 2) * 64:(b % 2) * 64 + 64, :]
    return out


def kernel(x, W_Q, W_K, W_V, W_O):
    x = np.asarray(x, np.float32)
    in_maps = make_in_maps(x, np.asarray(W_Q, np.float32),
                           np.asarray(W_K, np.float32),
                           np.asarray(W_V, np.float32),
                           np.asarray(W_O, np.float32))
    nc = _get_nc()
    res = run_bass_kernel_spmd(nc, in_maps, core_ids=list(range(NCORES)))
    return assemble(res.results)


# revision 48
# speedup vs baseline: 1.0407x; 1.0264x over previous
"""Causal multi-head attention (B=4, T=2048, D=1024, H=16) on 8 NeuronCores.

Sharding:
  stage 1 (QKV proj + attention): core c -> batch c%4, head-group c//4
    (8 of 16 heads, 512 of 1024 channels). Data-parallel on B, tensor-
    parallel on heads.
  stage 2 (output projection): FOUR AllToAlls, one per 512-token q-chunk.
    Core j owns rows {qc*512 + j*64 .. +64} for every batch; each collective
    re-shards one finished q-chunk so its output projection overlaps the
    next chunk's attention, and the tail after the last chunk is only one
    small collective + 1/4 of the O-projection.

Attention internals run in bf16 (Q^T, K^T, V, probabilities) which keeps
every matmul at 1 PE-cycle/row regardless of width. PV runs q-major:
out[128q, 65] = pt[128k,128q]^T-block-chain @ (V|ones), using the FULL 128
output partitions (the old k-major [65,512] layout wasted half the PE) and
yielding the softmax denominator per-partition, so normalization is a
single per-partition DVE divide - no PE broadcast matmuls. The [q,ch]->
[ch,q] flip needed by the O-projection happens for free inside the
post-collective read via DMA xbar transposes (dma_start_transpose).

Projections stay fp32r (accuracy); psum->sbuf copies convert to bf16.
The t-chunk loop interleaves projections/O-projections with attention as
PE filler work, with deadline-aware pacing so chunk-3's K/V projections
land inside qc3's ACT-bound window without stalling its diagonal chains.
"""
import numpy as np
import ml_dtypes

import concourse.bass as bass
import concourse.mybir as mybir
import concourse.tile as tile
from concourse.tile import add_dep_helper
from concourse.bass_utils import run_bass_kernel_spmd

F32 = mybir.dt.float32
F32R = mybir.dt.float32r
BF16 = mybir.dt.bfloat16
EXP = mybir.ActivationFunctionType.Exp

P = 128
B, T, D = 4, 2048, 1024
H, HD = 16, 64
NCORES = 8
CH = D // 2          # channels per core (8 heads)
NHP = 4              # head pairs per core
NKT = T // P         # 16 k-tiles
NQC = T // 512       # 4 q-chunks
NIT = D // P         # 8 input-dim tiles


def _split_multiwaits(nc) -> int:
    """walrus here rejects >1 sem wait per instruction; split extras into
    wait-only NoOps on the same engine."""
    nsplit = 0
    for f in nc.m.functions:
        for bb in f.blocks:
            if not any(
                i.sync_info is not None and i.sync_info.on_wait is not None
                and len(i.sync_info.on_wait) > 1 for i in bb.instructions
            ):
                continue
            new_list = []
            for inst in bb.instructions:
                si = inst.sync_info
                if si is not None and si.on_wait is not None and len(si.on_wait) > 1:
                    waits = list(si.on_wait)
                    for k, w in enumerate(waits[:-1]):
                        n = mybir.InstNoOp(
                            name=f"{inst.name}-wsplit{k}", ins=[], outs=[])
                        n.engine = inst.engine
                        n.sync_info = mybir.SyncInfo(on_wait=[w], on_update=[])
                        new_list.append(n)
                        nsplit += 1
                    inst.sync_info = mybir.SyncInfo(
                        on_wait=[waits[-1]], on_update=list(si.on_update or []))
                new_list.append(inst)
            bb.instructions = new_list
    return nsplit


def _build_nc(sim: bool = False, norm_mode: str = "recip",
              dbg: bool = False):
    nc = bass.Bass("TRN2", target_bir_lowering=False, debug=False,
                   num_devices=NCORES)
    xt_d = nc.dram_tensor("xt", [D, T], F32R, kind="ExternalInput").ap()
    wq_d = nc.dram_tensor("wq", [D, CH], F32R, kind="ExternalInput").ap()
    wk_d = nc.dram_tensor("wk", [D, CH], F32R, kind="ExternalInput").ap()
    wv_d = nc.dram_tensor("wv", [D, CH], F32R, kind="ExternalInput").ap()
    wo_d = nc.dram_tensor("wo", [D, D], BF16, kind="ExternalInput").ap()
    out_d = nc.dram_tensor("out", [NQC, 2, P, D], F32,
                           kind="ExternalOutput").ap()
    a2a_in = [nc.dram_tensor(f"a2a_in{m}", [NCORES, 64, CH], BF16).ap()
              for m in range(NQC)]
    a2a_out = [nc.dram_tensor(f"a2a_out{m}", [NCORES, 64, CH], BF16).ap()
               for m in range(NQC)]
    if dbg:  # dbg=1: tensors + copies; dbg=2: tensors only (layout probe)
        dbg_in = [nc.dram_tensor(f"dbg_in{m}", [NCORES, 64, CH], BF16,
                                 kind="ExternalOutput").ap()
                  for m in range(NQC)]
        dbg_out = [nc.dram_tensor(f"dbg_out{m}", [NCORES, 64, CH], BF16,
                                  kind="ExternalOutput").ap()
                   for m in range(NQC)]
    a2a_in_r = [a.rearrange("s q c -> (s q) c") for a in a2a_in]
    a2a_out_r = [a.rearrange("s q c -> (s q) c") for a in a2a_out]

    scale = float(1.0 / np.sqrt(HD))

    with tile.TileContext(nc) as tc:
        with (
            tc.tile_pool(name="persist", bufs=1) as persist,
        ):
            # ---- persistent SBUF tensors -------------------------------
            kt_s = persist.tile([P, NHP, T], BF16)              # K^T
            # V | ones | pad: 66-element slots keep every head-slot 4-byte
            # aligned (65 x 2B = 130B slots corrupt packed bf16 DVE writes
            # on hardware) and give the ones column a private 32-bit word
            va = persist.tile([P, NKT, NHP, 2, HD + 2], BF16)
            pt = persist.tile([P, NKT, 2, 512], BF16)           # probs
            wo16 = persist.tile([P, NIT, D], BF16)              # W_O^T

            with (
                tc.tile_pool(name="wpool", bufs=1) as wpool,
                tc.tile_pool(name="xpool", bufs=2) as xpool,
                tc.tile_pool(name="qpool", bufs=2) as qpool,
                tc.tile_pool(name="mpool", bufs=1) as mpool,
                tc.tile_pool(name="aoq_pool", bufs=8) as aoq_pool,
                tc.tile_pool(name="aob_pool", bufs=16) as aob_pool,
                tc.tile_pool(name="osb_pool", bufs=2) as osb_pool,
                tc.tile_pool(name="rpool", bufs=4) as rpool,
                tc.tile_pool(name="ppool", bufs=2, space="PSUM") as ppool,
                tc.tile_pool(name="ps_s", bufs=2, space="PSUM") as ps_s,
                tc.tile_pool(name="ps_pv", bufs=2, space="PSUM") as ps_pv,
            ):
                wq = wpool.tile([P, NIT, CH], F32R)
                wk = wpool.tile([P, NIT, CH], F32R)
                wv = wpool.tile([P, NIT, CH], F32R)
                xt_r = xt_d.rearrange("(i p) t -> p i t", p=P)
                wo_r = wo_d.rearrange("(i p) o -> p i o", p=P)
                xtc0 = xpool.tile([P, NIT, 512], F32R, tag="xtc")
                for it in range(NIT):
                    nc.sync.dma_start(wv[:, it], wv_d.rearrange(
                        "(i p) o -> p i o", p=P)[:, it])
                    nc.sync.dma_start(xtc0[:, it], xt_r[:, it, 0:512])
                for it in range(NIT):
                    nc.sync.dma_start(wq[:, it], wq_d.rearrange(
                        "(i p) o -> p i o", p=P)[:, it])
                    nc.sync.dma_start(wk[:, it], wk_d.rearrange(
                        "(i p) o -> p i o", p=P)[:, it])

                # ones column of V (softmax denominator source); on DVE so
                # it serializes with the V psum->sbuf copies
                nc.vector.memset(va[:, :, :, :, HD], 1.0)
                masks = []
                for i in range(4):
                    m = mpool.tile([P, 512], BF16, tag=f"mask{i}")
                    nc.gpsimd.memset(m[:], 1.0)
                    nc.gpsimd.affine_select(
                        out=m[:], in_=m[:],
                        compare_op=mybir.AluOpType.is_ge,
                        fill=0.0, base=-P * i, channel_multiplier=-1,
                        pattern=[[1, 512]])
                    masks.append(m)

                # ---- filler queue: [fn, deadline_unit_or_None] ---------
                pending = []
                unit_ctr = [0]
                filler_acc = [0.0]

                def emit_fillers(remaining):
                    u = unit_ctr[0]
                    i = 0
                    while i < len(pending):
                        if pending[i][1] is not None and pending[i][1] <= u:
                            pending.pop(i)[0]()
                        else:
                            i += 1
                    if not pending:
                        return
                    filler_acc[0] += len(pending) / max(1, remaining)
                    while filler_acc[0] >= 1.0 and pending:
                        filler_acc[0] -= 1.0
                        pending.pop(0)[0]()

                def drain_pending():
                    while pending:
                        pending.pop(0)[0]()

                # ---- projections --------------------------------------
                def project(tc4, xtc=None):
                    """Build QKV projection group closures for chunk tc4.
                    Returns (qtc, qgroups, kgroups, vgroups)."""
                    if xtc is None:
                        xtc = xpool.tile([P, NIT, 512], F32R, tag="xtc")
                        for it in range(NIT):
                            nc.sync.dma_start(
                                xtc[:, it],
                                xt_r[:, it, tc4 * 512:(tc4 + 1) * 512])
                    qtc = qpool.tile([P, NHP, 512], BF16, tag="qtc")

                    # groups are split in halves: finer filler pacing
                    # lets PE work drip into the ~500ns diag-unit stalls
                    def qk_group(w, dst, dsl, ot):
                        box = {}

                        def g1():
                            box["ps"] = ppool.tile([P, 512], F32, tag="proj")
                            for it in range(4):
                                nc.tensor.matmul(
                                    box["ps"][:], w[:, it, ot * P:(ot + 1) * P],
                                    xtc[:, it], start=(it == 0), stop=False)

                        def g2():
                            ps = box["ps"]
                            for it in range(4, NIT):
                                nc.tensor.matmul(
                                    ps[:], w[:, it, ot * P:(ot + 1) * P],
                                    xtc[:, it], start=False,
                                    stop=(it == NIT - 1))
                            nc.vector.tensor_copy(dst[:, ot, dsl], ps[:])
                        return g1, g2

                    def v_group(tt4):
                        box = {}

                        def g1():
                            box["ps"] = ppool.tile([P, 512], F32, tag="proj")
                            for it in range(4):
                                nc.tensor.matmul(
                                    box["ps"][:],
                                    xtc[:, it, tt4 * P:(tt4 + 1) * P],
                                    wv[:, it], start=(it == 0), stop=False)

                        def g2():
                            ps = box["ps"]
                            for it in range(4, NIT):
                                nc.tensor.matmul(
                                    ps[:], xtc[:, it, tt4 * P:(tt4 + 1) * P],
                                    wv[:, it], start=False,
                                    stop=(it == NIT - 1))
                            nc.vector.tensor_copy(
                                va[:, tc4 * 4 + tt4, :, :, 0:HD],
                                ps[:].rearrange("p (hp h d) -> p hp h d",
                                                hp=NHP, h=2))
                        return g1, g2

                    qg = [h for ot in range(NHP)
                          for h in qk_group(wq, qtc, slice(0, 512), ot)]
                    kg = [h for ot in range(NHP)
                          for h in qk_group(
                              wk, kt_s,
                              slice(tc4 * 512, (tc4 + 1) * 512), ot)]
                    vg = [h for tt4 in range(4) for h in v_group(tt4)]
                    return qtc, qg, kg, vg

                # chunk 0: V first via 4 concurrent psums (it-major) so each
                # arriving (x, wv) DMA chunk feeds 4 matmuls during the
                # DMA-bound startup ramp
                qtc, qg0, kg0, vg0 = project(0, xtc=xtc0)

                def v_block0():
                    pss = [
                        ppool.tile([P, 512], F32, tag="proj", name="v0ps0"),
                        ppool.tile([P, 512], F32, tag="proj", name="v0ps1"),
                        ps_pv.tile([P, 512], F32, tag="pv", name="v0ps2"),
                        ps_pv.tile([P, 512], F32, tag="pv", name="v0ps3"),
                    ]
                    for it in range(NIT):
                        for tt4 in range(4):
                            nc.tensor.matmul(
                                pss[tt4][:],
                                xtc0[:, it, tt4 * P:(tt4 + 1) * P],
                                wv[:, it], start=(it == 0),
                                stop=(it == NIT - 1))
                    for tt4 in range(4):
                        nc.vector.tensor_copy(
                            va[:, tt4, :, :, 0:HD],
                            pss[tt4][:].rearrange(
                                "p (hp h d) -> p hp h d", hp=NHP, h=2))

                v_block0()
                for ot in range(NHP):
                    qg0[2 * ot]()
                    qg0[2 * ot + 1]()
                    kg0[2 * ot]()
                    kg0[2 * ot + 1]()

                # ---- collectives / stage 2 -----------------------------
                # Tile does not track DRAM-tensor data flow, so the
                # aoq-DMA -> collective -> transposed-read chain needs
                # explicit dependencies.
                aob_map = {}
                aoq_dmas = {m: [] for m in range(NQC)}
                cc_insts = {}

                def emit_collective(m):
                    if sim:
                        cc = nc.sync.dma_start(a2a_out[m], a2a_in[m])
                    else:
                        cc = nc.gpsimd.collective_compute(
                            "AllToAll", mybir.AluOpType.bypass,
                            replica_groups=[list(range(NCORES))],
                            ins=[a2a_in[m]], outs=[a2a_out[m]])
                    for d in aoq_dmas[m]:
                        add_dep_helper(cc.ins, d.ins,
                                       reason="collective reads a2a_in")
                    cc_insts[m] = cc

                tri = mpool.tile([P, P], BF16)
                nc.gpsimd.memset(tri[:], 0.0)
                nc.gpsimd.affine_select(
                    out=tri[:], in_=tri[:],
                    compare_op=mybir.AluOpType.is_ge,
                    fill=-240.0, base=0, channel_multiplier=-1,
                    pattern=[[1, P]])
                identity = mpool.tile([P, P], BF16)
                nc.gpsimd.memset(identity[:], 1.0)
                nc.gpsimd.affine_select(
                    out=identity[:], in_=identity[:],
                    compare_op=mybir.AluOpType.is_ge,
                    fill=0.0, base=0, channel_multiplier=-1,
                    pattern=[[1, P]])
                nc.gpsimd.affine_select(
                    out=identity[:], in_=identity[:],
                    compare_op=mybir.AluOpType.is_ge,
                    fill=0.0, base=0, channel_multiplier=1,
                    pattern=[[-1, P]])

                araw_box = {}

                def transpose_group(m, f, use_act):
                    """One fused DMA read of the q-major collective output
                    (first group), then 4 PE transposes per row-block f into
                    ch-major aob tiles (the DMA-xbar transposed read raced on
                    HW). use_act alternates copies over ACT+DVE - only worth
                    it at the tail when ACT is idle."""
                    g, half = f // 2, f % 2
                    def gfn():
                        if f == 0:
                            araw_box[m] = osb_pool.tile(
                                [P, 4, 512], BF16, tag="araw",
                                name=f"araw{m}")
                            d = nc.sync.dma_start(
                                araw_box[m][:],
                                a2a_out_r[m].rearrange(
                                    "(f p) c -> p f c", p=P))
                            add_dep_helper(d.ins, cc_insts[m].ins,
                                           reason="aob reads a2a_out")
                        araw = araw_box[m]
                        for cs in range(4):
                            tp = ppool.tile([P, 1024], BF16, tag="proj",
                                            name=f"tp{m}_{f}_{cs}")
                            nc.tensor.matmul(
                                tp[:, 0:P], araw[:, f, cs * P:(cs + 1) * P],
                                identity[:], start=True, stop=True,
                                is_transpose=True)
                            dst = aob_map[m][g * 4 + cs][:, half * P:
                                                         (half + 1) * P]
                            if use_act and cs % 2 == 1:
                                nc.scalar.copy(dst, tp[:, 0:P])
                            else:
                                nc.vector.tensor_copy(dst, tp[:, 0:P])
                    return gfn

                def queue_transposes(m, deadlines=(3, 7, 11, 15),
                                     use_act=False):
                    aob_map[m] = [aob_pool.tile([P, 256], BF16, tag="aob",
                                                name=f"aob{m}_{ct}")
                                  for ct in range(NIT)]
                    for f in range(4):
                        pending.append([transpose_group(m, f, use_act),
                                        deadlines[f]])

                wo_dmas = []

                def o_group(m, tt, oc, osb):
                    def g():
                        ps = ppool.tile([P, 512], F32, tag="proj")
                        for ct in range(NIT):
                            mm = nc.tensor.matmul(
                                ps[:], aob_map[m][ct][:, tt * P:(tt + 1) * P],
                                wo16[:, ct, oc * 512:(oc + 1) * 512],
                                start=(ct == 0), stop=(ct == NIT - 1))
                            # Tile misses the RAW dep on the moving operand
                            # for some DMA-written tiles; make it explicit.
                            add_dep_helper(mm.ins, wo_dmas[ct].ins,
                                           reason="oproj reads wo16")
                        nc.vector.tensor_copy(
                            osb[:, oc * 512:(oc + 1) * 512], ps[:])
                        nc.sync.dma_start(
                            out_d[m, tt][:, oc * 512:(oc + 1) * 512],
                            osb[:, oc * 512:(oc + 1) * 512])
                    return g

                def queue_oproj(m):
                    for tt in range(2):
                        osb = osb_pool.tile([P, D], F32, tag="osb")
                        pending.append([o_group(m, tt, 0, osb), None])
                        pending.append([o_group(m, tt, 1, osb), None])

                def wo_unit():
                    for it in range(NIT):
                        wo_dmas.append(
                            nc.sync.dma_start(wo16[:, it], wo_r[:, it]))

                # ---- attention -----------------------------------------
                def attend(hp, qc, qtc, aoq_tiles):
                    nkt = 4 * (qc + 1)
                    s2s = {}

                    def qk(kt):
                        ksl = slice(kt * P, (kt + 1) * P)
                        di = kt - 4 * qc
                        f0 = max(0, di) * P
                        diag = di >= 0
                        s2 = ps_s.tile([P, 1024], F32, tag="s2")
                        nc.tensor.matmul(s2[:, f0:512], kt_s[0:64, hp, ksl],
                                         qtc[0:64, hp, f0:],
                                         start=True, stop=not diag)
                        if diag:
                            # -240 strict upper triangle accumulated into the
                            # causal boundary block: exp then yields ~1e-13,
                            # replacing the DVE mask multiply (and its
                            # ACT->DVE->PE sem chain ahead of the PV chains)
                            nc.tensor.matmul(s2[:, f0:f0 + P], identity[:],
                                             tri[:], start=False, stop=True)
                        nc.tensor.matmul(s2[:, 512 + f0:1024],
                                         kt_s[64:128, hp, ksl],
                                         qtc[64:128, hp, f0:],
                                         start=True, stop=not diag)
                        if diag:
                            nc.tensor.matmul(s2[:, 512 + f0:512 + f0 + P],
                                             identity[:], tri[:],
                                             start=False, stop=True)
                        s2s[kt] = s2

                    def soft(kt):
                        s2 = s2s.pop(kt)
                        di = kt - 4 * qc
                        f0 = max(0, di) * P
                        ptv = pt[:, kt]
                        if f0 > 0:
                            s2v = s2[:].rearrange("p (a b) -> p a b", a=2)
                            nc.scalar.activation(
                                ptv[:, :, f0:], s2v[:, :, f0:], EXP,
                                scale=scale)
                        else:
                            nc.scalar.activation(
                                ptv.rearrange("p a b -> p (a b)"), s2[:],
                                EXP, scale=scale)
                        if di >= 0:
                            qb = di
                            for h in range(2):
                                pv = ps_pv.tile([P, 512], F32, tag="pv")
                                for kt2 in range(kt + 1):
                                    nc.tensor.matmul(
                                        pv[:, 0:65],
                                        pt[:, kt2, h, qb * P:(qb + 1) * P],
                                        va[:, kt2, hp, h, 0:HD + 1],
                                        start=(kt2 == 0), stop=(kt2 == kt))
                                dst = aoq_tiles[qb][:, hp, h]
                                if norm_mode == "div":
                                    nc.vector.tensor_tensor(
                                        dst, pv[:, 0:64],
                                        pv[:, 64:65].to_broadcast((P, 64)),
                                        op=mybir.AluOpType.divide)
                                else:
                                    rden = rpool.tile([P, 1], F32, tag="rd")
                                    nc.vector.reciprocal(rden[:], pv[:, 64:65])
                                    nc.vector.tensor_mul(
                                        dst, pv[:, 0:64],
                                        rden[:].to_broadcast((P, 64)))
                            if hp == NHP - 1:
                                d = nc.sync.dma_start(
                                    a2a_in_r[qc][qb * P:(qb + 1) * P],
                                    aoq_tiles[qb][:])
                                aoq_dmas[qc].append(d)
                        unit_ctr[0] += 1
                        emit_fillers((nkt - 1 - kt) + (NHP - 1 - hp) * nkt)

                    qk(0)
                    for kt in range(1, nkt):
                        qk(kt)
                        soft(kt - 1)
                    soft(nkt - 1)

                # ---- main interleaved loop -----------------------------
                nxt = None
                for tc4 in range(NQC):
                    unit_ctr[0] = 0
                    nkt = 4 * (tc4 + 1)
                    if tc4 == 0:
                        pending.append([wo_unit, None])
                        nxt = project(1)
                        for ot in range(NHP):
                            pending.append([nxt[1][2 * ot], None])      # Q
                            pending.append([nxt[1][2 * ot + 1], None])
                            pending.append([nxt[2][2 * ot], None])      # K
                            pending.append([nxt[2][2 * ot + 1], None])
                        for h in nxt[3]:
                            pending.append([h, None])            # V
                    elif tc4 == 1:
                        queue_transposes(0)
                        nxt = project(2)
                        for ot in range(NHP):
                            pending.append([nxt[1][2 * ot], None])
                            pending.append([nxt[1][2 * ot + 1], None])
                            pending.append([nxt[2][2 * ot], None])
                            pending.append([nxt[2][2 * ot + 1], None])
                        for h in nxt[3]:
                            pending.append([h, None])
                        queue_oproj(0)
                    elif tc4 == 2:
                        queue_transposes(1)
                        nxt = project(3)
                        for h in nxt[1]:
                            pending.append([h, None])   # Q only
                        queue_oproj(1)
                    elif tc4 == 3:
                        # K/V of chunk 3 land inside qc3's window, deadline'd
                        # ahead of the attention instructions that read them
                        # (PE is in-order: a chain emitted before its V-group
                        # would deadlock).
                        queue_transposes(2)
                        pending.append([nxt[2][0], 7])            # K0
                        pending.append([nxt[2][1], 8])
                        for tt4 in range(4):
                            pending.append([nxt[3][2 * tt4], 8 + tt4])   # V
                            pending.append([nxt[3][2 * tt4 + 1], 9 + tt4])
                        for ot in range(1, NHP):
                            pending.append([nxt[2][2 * ot], ot * 16 + 7])
                            pending.append([nxt[2][2 * ot + 1], ot * 16 + 8])
                        queue_oproj(2)

                    aoq_tiles = [aoq_pool.tile([P, NHP, 2, HD], BF16,
                                               tag="aoq", name=f"aoq{tc4}_{i}")
                                 for i in range(4)]
                    for hp in range(NHP):
                        attend(hp, tc4, qtc, aoq_tiles)

                    # non-deadline stragglers must not leak past the window
                    # (next chunk's attention reads their outputs)
                    i = 0
                    while i < len(pending):
                        if pending[i][1] is None:
                            pending.pop(i)[0]()
                        else:
                            i += 1

                    emit_collective(tc4)
                    if dbg == 1:
                        d1 = nc.sync.dma_start(dbg_in[tc4], a2a_in[tc4])
                        d2 = nc.sync.dma_start(dbg_out[tc4], a2a_out[tc4])
                        add_dep_helper(d1.ins, cc_insts[tc4].ins, reason="dbg")
                        add_dep_helper(d2.ins, cc_insts[tc4].ins, reason="dbg")
                    if tc4 < 3:
                        qtc = nxt[0]

                # ---- tail: last O-projection ---------------------------
                drain_pending()
                queue_transposes(3, deadlines=(0, 0, 0, 0),
                                 use_act=True)
                queue_oproj(3)
                drain_pending()
                if dbg == 5:
                    dump0 = nc.dram_tensor(
                        "dump0", [NCORES, 64, CH], BF16,
                        kind="ExternalOutput").ap()
                    nc.sync.dma_start(dump0, a2a_in[0])
                if dbg == 6:
                    dump0 = nc.dram_tensor(
                        "dump0", [512, 256], F32R,
                        kind="ExternalOutput").ap()
                    nc.sync.dma_start(dump0, xt_d[0:512, 0:256])
                if dbg == 4:
                    vadump = nc.dram_tensor(
                        "vadump", [P, NKT, NHP, 2, HD + 2], BF16,
                        kind="ExternalOutput").ap()
                    ptdump = nc.dram_tensor(
                        "ptdump", [P, NKT, 2, 512], BF16,
                        kind="ExternalOutput").ap()
                    a2adump = nc.dram_tensor(
                        "a2adump", [NCORES, 64, CH], BF16,
                        kind="ExternalOutput").ap()
                    nc.sync.dma_start(vadump, va[:])
                    nc.sync.dma_start(ptdump, pt[:])
                    nc.sync.dma_start(a2adump, a2a_in[0])
                if dbg == 3:
                    # end-of-kernel snapshots: after all consumers, so they
                    # cannot mask ordering bugs
                    for m in range(NQC):
                        d1 = nc.sync.dma_start(dbg_in[m], a2a_in[m])
                        d2 = nc.sync.dma_start(dbg_out[m], a2a_out[m])
                        add_dep_helper(d1.ins, cc_insts[m].ins, reason="dbg")
                        add_dep_helper(d2.ins, cc_insts[m].ins, reason="dbg")

    _split_multiwaits(nc)
    return nc


_NC_CACHE = None


def _get_nc():
    global _NC_CACHE
    if _NC_CACHE is None:
        _NC_CACHE = _build_nc()
    return _NC_CACHE


def make_in_maps(x, W_Q, W_K, W_V, W_O):
    wqt = np.ascontiguousarray(W_Q.T)
    wkt = np.ascontiguousarray(W_K.T)
    wvt = np.ascontiguousarray(W_V.T)
    wot = np.ascontiguousarray(W_O.T).astype(ml_dtypes.bfloat16)
    in_maps = []
    for c in range(NCORES):
        g, b = c // 4, c % 4
        in_maps.append({
            "xt": np.ascontiguousarray(x[b].T),
            "wq": np.ascontiguousarray(wqt[:, g * CH:(g + 1) * CH]),
            "wk": np.ascontiguousarray(wkt[:, g * CH:(g + 1) * CH]),
            "wv": np.ascontiguousarray(wvt[:, g * CH:(g + 1) * CH]),
            "wo": wot,
        })
    return in_maps


def assemble(results):
    out = np.empty((B, T, D), np.float32)
    for j in range(NCORES):
        o = results[j]["out"]  # [NQC, 2, 128, D]
        for qc in range(NQC):
            for b in range(B):
                r0 = qc * 512 + j * 64
                out[b, r0:r0 + 64, :] = o[qc, b // 2,
                                          (b % 2) * 64:(b % 2) * 64 + 64, :]
    return out


def kernel(x, W_Q, W_K, W_V, W_O):
    x = np.asarray(x, np.float32)
    in_maps = make_in_maps(x, np.asarray(W_Q, np.float32),
                           np.asarray(W_K, np.float32),
                           np.asarray(W_V, np.float32),
                           np.asarray(W_O, np.float32))
    nc = _get_nc()
    res = run_bass_kernel_spmd(nc, in_maps, core_ids=list(range(NCORES)))
    return assemble(res.results)
